# revision 2
# baseline (speedup 1.0000x reference)
# Trainium2 Bass kernel for nn_AttentionModule_16011638080155.
#
# Reference computation (see problem): cross-attention with length-normalized
# RoPE, softmax over context L, out-projection, written as [B, D_MODEL, T].
#
# Sharding: 8 cores = (batch b in 0..4) x (T half in 0..2). Each core computes
# its full attention output slice [D_MODEL, 1024] independently (k/v projection
# duplicated across the two T-halves of a batch; no collectives).
#
# Device layout (per core), everything "S-transposed" so softmax output feeds
# the PV matmul with no transposes:
#   q.T  [a=512, t=1024]  = WqT.T @ x        (a = attn dim, heads-major)
#   k.T  [a=512, l=2048]  = WkT.T @ ctxT
#   v    [l=2048, a=512]  = ctxT.T @ WvT     (stored per l-tile, ones-augmented)
#   S.T  [l, t]           = k_rope.T x q_rope (per head, row-tiled head pairs)
#   P.T  = exp(S.T / sqrt(512))              (ACT, per-partition mask bias)
#   O    [65, t] = [V | 1s].T @ P.T          (row 64 = softmax sums)
#   out  [dm, t] = WoT.T @ (O / sums)        (+bo via ACT bias)
import math
import os

import numpy as np

# ---------------------------------------------------------------------------
# Workaround for walrus CoreV2/V3 "Too many sync wait commands" on the Tile
# kernel-tail drain: move the accumulated sem waits off the single Drain
# instruction onto preceding nop instructions (same engine, in-order), at
# most 1 wait per instruction.
# ---------------------------------------------------------------------------


def _install_tile_drain_patch():
    import concourse.mybir as mybir
    import concourse.tile as tile_mod
    from concourse.vector_clock import ScopedClock

    if getattr(tile_mod.TileContext, "_drain_patch_installed", False):
        return

    def _patched_drain_and_barrier(self, tick_clock, wait_clock):
        nc = self.nc
        sink = nc.sync.nop(nofuse=True)
        wait_clock.add_sem_waits(
            sink.ins, ScopedClock({None: tick_clock.global_clock})
        )
        si = sink.ins.sync_info
        waits = list(si.on_wait) if si is not None else []
        if len(waits) > 1:
            sink.ins.sync_info = mybir.SyncInfo(on_wait=waits[:1], on_update=[])
            rest = waits[1:]
            for i in range(len(rest)):
                n2 = nc.sync.nop(nofuse=True)
                n2.ins.sync_info = mybir.SyncInfo(
                    on_wait=rest[i : i + 1], on_update=[]
                )
        nc.sync.drain()

        nc.all_engine_barrier()
        assert self.sems is not None
        popped = nc._tile_sem_poison_stack.pop()
        assert popped is self._sem_poison
        nc.clear_and_free_semaphores(list(self.sems.allocated().values()))
        nc.all_engine_barrier()

    tile_mod.TileContext._drain_and_barrier = _patched_drain_and_barrier
    tile_mod.TileContext._drain_patch_installed = True


# ---------------------------------------------------------------------------
# Problem constants (hardcoded per the harness contract).
# ---------------------------------------------------------------------------
B = 4
D_MODEL = 512
T = 2048
L = 2048
D_CTX = 512
ATT = 512
H = 8
HD = 64
ROPE_GAMMA = 10.0
SCALE = math.sqrt(ATT)

N_CORES = 8
T_CORE = T // 2  # 1024, each core handles half the query positions
N_TCH = T_CORE // 512  # 2 chunks of 512
N_LCH = L // 512  # 4
N_LT = L // 128  # 16
MASK_NEG = -60.0  # applied post-scale inside exp(); exp(-60) ~ 8.8e-27


def _build_nc(cfg):
    """Build the single-core Bass program (same program runs SPMD on 8 cores)."""
    import concourse.bacc as bacc
    import concourse.mybir as mybir
    import concourse.tile as tile
    from contextlib import ExitStack

    f32 = mybir.dt.float32
    f32r = mybir.dt.float32r
    f16 = mybir.dt.float16
    AF = mybir.ActivationFunctionType
    ALU = mybir.AluOpType

    def r(ap):
        return ap

    nc = bacc.Bacc("TRN2", target_bir_lowering=False, debug=False)

    # ---- DRAM parameters -------------------------------------------------
    x = nc.declare_dram_parameter("x", [D_MODEL, T_CORE], f32r, isOutput=False)
    ctxT = nc.declare_dram_parameter("ctxT", [D_CTX, L], f32r, isOutput=False)
    wqt = nc.declare_dram_parameter("wqt", [D_MODEL, ATT], f32r, isOutput=False)
    wqts = nc.declare_dram_parameter("wqts", [D_MODEL, ATT], f32r, isOutput=False)
    wkt = nc.declare_dram_parameter("wkt", [D_CTX, ATT], f32r, isOutput=False)
    wkts = nc.declare_dram_parameter("wkts", [D_CTX, ATT], f32r, isOutput=False)
    wvt = nc.declare_dram_parameter("wvt", [D_CTX, ATT], f32r, isOutput=False)
    wot = nc.declare_dram_parameter("wot", [ATT, D_MODEL], f32, isOutput=False)
    cq = nc.declare_dram_parameter("cq", [128, T_CORE], f32, isOutput=False)
    sq = nc.declare_dram_parameter("sq", [128, T_CORE], f32, isOutput=False)
    ck = nc.declare_dram_parameter("ck", [128, L], f32, isOutput=False)
    sk = nc.declare_dram_parameter("sk", [128, L], f32, isOutput=False)
    bo = nc.declare_dram_parameter("bo", [D_MODEL], f32, isOutput=False)
    if cfg["qk_bias"]:
        bqv = nc.declare_dram_parameter("bqv", [128, 8], f32, isOutput=False)
        # columns: [bq(4 m-tiles) | bq_swapped(4 m-tiles)] per-partition values
        bkv = nc.declare_dram_parameter("bkv", [128, 8], f32, isOutput=False)
    if cfg["v_bias"]:
        bvt = nc.declare_dram_parameter("bvt", [128, ATT], f32, isOutput=False)
    if cfg["kmask"]:
        kmb = nc.declare_dram_parameter("kmb", [128, N_LT], f32, isOutput=False)
    out = nc.declare_dram_parameter("out", [D_MODEL, T_CORE], f32, isOutput=True)

    x_re = x.rearrange("(kp p) t -> p kp t", p=128)
    ctxT_re = ctxT.rearrange("(kp p) l -> p kp l", p=128)
    wqt_re = wqt.rearrange("(kp p) a -> p kp a", p=128)
    wqts_re = wqts.rearrange("(kp p) a -> p kp a", p=128)
    wkt_re = wkt.rearrange("(kp p) a -> p kp a", p=128)
    wkts_re = wkts.rearrange("(kp p) a -> p kp a", p=128)
    wvt_re = wvt.rearrange("(kp p) a -> p kp a", p=128)
    bo_re = bo.rearrange("(kp p) -> p kp", p=128)
    out_re = out.rearrange("(kp p) t -> p kp t", p=128)

    with tile.TileContext(nc) as tc, ExitStack() as ctx:
        # ---- persistent SBUF tiles --------------------------------------
        per = ctx.enter_context(tc.tile_pool(name="per", bufs=1))
        qropeT = [per.tile([128, T_CORE], f16, tag=f"qrope{m}", name=f"qrope{m}") for m in range(4)]
        kropeT = [per.tile([128, L], f16, tag=f"krope{m}", name=f"krope{m}") for m in range(4)]
        vaug = [per.tile([128, H * 65], f16, tag=f"vaug{lt}", name=f"vaug{lt}") for lt in range(N_LT)]
        onorm = [
            [per.tile([64, 512], f16, tag=f"on{h}_{tch}", name=f"on{h}_{tch}") for tch in range(N_TCH)]
            for h in range(H)
        ]
        wot_sb = [per.tile([64, D_MODEL], f16, tag=f"wot{h}", name=f"wot{h}") for h in range(H)]
        bo_sb = per.tile([128, 4], f32, tag="bo")
        ones1 = per.tile([1, 64], f32, tag="ones1")
        nc.vector.memset(ones1[:], 1.0)
        if cfg["kmask"]:
            kmb_sb = per.tile([128, N_LT], f32, tag="kmb")
            nc.sync.dma_start(kmb_sb[:], kmb[:])

        wotf = [
            per.tile([64, D_MODEL], f32, tag=f"wotf{h}", name=f"wotf{h}")
            for h in range(H)
        ]
        for h in range(H):
            nc.sync.dma_start(wotf[h][:], wot[64 * h : 64 * h + 64, :])
            nc.vector.tensor_copy(wot_sb[h][:], wotf[h][:])
        nc.sync.dma_start(bo_sb[:], bo_re)

        # ---- phase Q: q.T projection + rope -----------------------------
        with tc.tile_pool(name="qph", bufs=1) as qph, tc.tile_pool(
            name="qpsum", bufs=2, space="PSUM"
        ) as qpsum, tc.tile_pool(name="qtmp", bufs=4) as qtmp:
            x_sb = qph.tile([128, 4, T_CORE], f32r, tag="x")
            wq_sb = qph.tile([128, 4, ATT], f32r, tag="wq")
            wqs_sb = qph.tile([128, 4, ATT], f32r, tag="wqs")
            cq_sb = qph.tile([128, T_CORE], f32, tag="cq")
            sq_sb = qph.tile([128, T_CORE], f32, tag="sq")
            nc.sync.dma_start(x_sb[:], x_re)
            nc.sync.dma_start(wq_sb[:], wqt_re)
            nc.sync.dma_start(wqs_sb[:], wqts_re)
            nc.sync.dma_start(cq_sb[:], cq[:])
            nc.sync.dma_start(sq_sb[:], sq[:])
            if cfg["qk_bias"]:
                bq_sb = qph.tile([128, 8], f32, tag="bq")
                nc.sync.dma_start(bq_sb[:], bqv[:])
                ones_t = qph.tile([1, 512], f32, tag="onest")
                nc.vector.memset(ones_t[:], 1.0)

            for m in range(4):
                for tch in range(N_TCH):
                    ts = slice(512 * tch, 512 * (tch + 1))
                    pc = qpsum.tile([128, 512], f32, tag="pc")
                    ps = qpsum.tile([128, 512], f32, tag="ps")
                    for k in range(4):
                        nc.tensor.matmul(
                            pc[:],
                            r(wq_sb[:, k, 128 * m : 128 * (m + 1)]),
                            r(x_sb[:, k, ts]),
                            start=(k == 0),
                            stop=(k == 3) and not cfg["qk_bias"],
                        )
                        nc.tensor.matmul(
                            ps[:],
                            r(wqs_sb[:, k, 128 * m : 128 * (m + 1)]),
                            r(x_sb[:, k, ts]),
                            start=(k == 0),
                            stop=(k == 3) and not cfg["qk_bias"],
                        )
                    if cfg["qk_bias"]:
                        nc.tensor.matmul(
                            pc[:], r(bq_sb[:, m : m + 1]), r(ones_t[:]),
                            start=False, stop=True,
                        )
                        nc.tensor.matmul(
                            ps[:], r(bq_sb[:, 4 + m : 5 + m]), r(ones_t[:]),
                            start=False, stop=True,
                        )
                    t1 = qtmp.tile([128, 512], f32, tag="t1")
                    t2 = qtmp.tile([128, 512], f32, tag="t2")
                    nc.vector.tensor_tensor(t1[:], pc[:], cq_sb[:, ts], ALU.mult)
                    nc.vector.tensor_tensor(t2[:], ps[:], sq_sb[:, ts], ALU.mult)
                    nc.vector.tensor_tensor(
                        qropeT[m][:, ts], t1[:], t2[:], ALU.add
                    )

        # ---- phase KV: k.T projection + rope, v projection --------------
        with tc.tile_pool(name="kph", bufs=1) as kph, tc.tile_pool(
            name="kstream", bufs=2
        ) as kstream, tc.tile_pool(name="kpsum", bufs=2, space="PSUM") as kpsum, \
                tc.tile_pool(name="ktmp", bufs=4) as ktmp:
            wk_sb = kph.tile([128, 4, ATT], f32r, tag="wk")
            wks_sb = kph.tile([128, 4, ATT], f32r, tag="wks")
            wv_sb = kph.tile([128, 4, ATT], f32r, tag="wv")
            nc.sync.dma_start(wk_sb[:], wkt_re)
            nc.sync.dma_start(wks_sb[:], wkts_re)
            nc.sync.dma_start(wv_sb[:], wvt_re)
            if cfg["qk_bias"]:
                bk_sb = kph.tile([128, 8], f32, tag="bk")
                nc.sync.dma_start(bk_sb[:], bkv[:])
                ones_l = kph.tile([1, 512], f32, tag="onesl")
                nc.vector.memset(ones_l[:], 1.0)
            if cfg["v_bias"]:
                bv_sb = kph.tile([128, ATT], f32, tag="bv")
                nc.sync.dma_start(bv_sb[:], bvt[:])

            for lch in range(N_LCH):
                ls = slice(512 * lch, 512 * (lch + 1))
                ctx_sb = kstream.tile([128, 4, 512], f32r, tag="ctxs")
                ck_sb = kstream.tile([128, 512], f32, tag="cks")
                sk_sb = kstream.tile([128, 512], f32, tag="sks")
                nc.sync.dma_start(ctx_sb[:], ctxT_re[:, :, ls])
                nc.sync.dma_start(ck_sb[:], ck[:, ls])
                nc.sync.dma_start(sk_sb[:], sk[:, ls])

                # k.T for this l chunk: all 4 a-tiles
                for m in range(4):
                    pc = kpsum.tile([128, 512], f32, tag="kc")
                    ps = kpsum.tile([128, 512], f32, tag="ks")
                    for k in range(4):
                        nc.tensor.matmul(
                            pc[:],
                            r(wk_sb[:, k, 128 * m : 128 * (m + 1)]),
                            r(ctx_sb[:, k, :]),
                            start=(k == 0),
                            stop=(k == 3) and not cfg["qk_bias"],
                        )
                        nc.tensor.matmul(
                            ps[:],
                            r(wks_sb[:, k, 128 * m : 128 * (m + 1)]),
                            r(ctx_sb[:, k, :]),
                            start=(k == 0),
                            stop=(k == 3) and not cfg["qk_bias"],
                        )
                    if cfg["qk_bias"]:
                        nc.tensor.matmul(
                            pc[:], r(bk_sb[:, m : m + 1]), r(ones_l[:]),
                            start=False, stop=True,
                        )
                        nc.tensor.matmul(
                            ps[:], r(bk_sb[:, 4 + m : 5 + m]), r(ones_l[:]),
                            start=False, stop=True,
                        )
                    t1 = ktmp.tile([128, 512], f32, tag="kt1")
                    t2 = ktmp.tile([128, 512], f32, tag="kt2")
                    nc.vector.tensor_tensor(t1[:], pc[:], ck_sb[:], ALU.mult)
                    nc.vector.tensor_tensor(t2[:], ps[:], sk_sb[:], ALU.mult)
                    nc.vector.tensor_tensor(
                        kropeT[m][:, ls], t1[:], t2[:], ALU.add
                    )

                # v for the 4 l-tiles of this chunk
                for j in range(4):
                    lt = 4 * lch + j
                    pv = kpsum.tile([128, 512], f32, tag="vps")
                    for k in range(4):
                        nc.tensor.matmul(
                            pv[:],
                            r(ctx_sb[:, k, 128 * j : 128 * (j + 1)]),
                            r(wv_sb[:, k, :]),
                            start=(k == 0),
                            stop=(k == 3),
                        )
                    if cfg["v_bias"]:
                        nc.vector.tensor_tensor(pv[:], pv[:], bv_sb[:], ALU.add)
                    va = vaug[lt][:].rearrange("p (h e) -> p h e", e=65)
                    nc.vector.tensor_copy(
                        va[:, :, 0:64],
                        pv[:].rearrange("p (h d) -> p h d", h=H),
                    )
                    nc.vector.memset(va[:, :, 64], 1.0)

        # ---- attention ---------------------------------------------------
        with tc.tile_pool(name="apsum", bufs=2, space="PSUM") as apsum, \
                tc.tile_pool(name="opsum", bufs=2, space="PSUM") as opsum, \
                                tc.tile_pool(name="ptile", bufs=4) as ptile, \
                tc.tile_pool(name="stile", bufs=6) as stile:
            for tch in range(N_TCH):
                ts = slice(512 * tch, 512 * (tch + 1))
                for hp in range(4):
                    hA, hB = 2 * hp, 2 * hp + 1
                    oA = opsum.tile([65, 512], f32, tag="oA")
                    oB = opsum.tile([65, 512], f32, tag="oB")
                    for lt in range(N_LT):
                        lw = slice(128 * lt, 128 * (lt + 1))
                        g = apsum.tile([128, 1024], f32, tag="g")
                        nc.tensor.matmul(
                            g[:, 0:512],
                            r(kropeT[hp][0:64, lw]),
                            r(qropeT[hp][0:64, ts]),
                            start=True, stop=True,
                            tile_position=(0, 0),
                        )
                        nc.tensor.matmul(
                            g[:, 512:1024],
                            r(kropeT[hp][64:128, lw]),
                            r(qropeT[hp][64:128, ts]),
                            start=True, stop=True,
                            tile_position=(64, 0),
                        )
                        pT = ptile.tile([128, 1024], f16, tag="pT")
                        if cfg["kmask"]:
                            nc.scalar.activation(
                                pT[:], g[:], AF.Exp,
                                bias=kmb_sb[:, lt : lt + 1],
                                scale=1.0 / SCALE,
                            )
                        else:
                            nc.scalar.activation(
                                pT[:], g[:], AF.Exp, scale=1.0 / SCALE
                            )
                        nc.tensor.matmul(
                            oA[:],
                            r(vaug[lt][:, 65 * hA : 65 * hA + 65]),
                            r(pT[:, 0:512]),
                            start=(lt == 0), stop=(lt == N_LT - 1),
                        )
                        nc.tensor.matmul(
                            oB[:],
                            r(vaug[lt][:, 65 * hB : 65 * hB + 65]),
                            r(pT[:, 512:1024]),
                            start=(lt == 0), stop=(lt == N_LT - 1),
                        )
                    # normalize: row 64 of oA/oB holds the softmax sums
                    for h, o in ((hA, oA), (hB, oB)):
                        srow = stile.tile([1, 512], f32, tag="srow")
                        nc.vector.tensor_copy(srow[:], o[64:65, :])
                        rrow = stile.tile([1, 512], f32, tag="rrow")
                        nc.vector.reciprocal(rrow[:], srow[:])
                        rs = stile.tile([64, 512], f32, tag="rs")
                        nc.gpsimd.partition_broadcast(rs[:], rrow[:])
                        nc.vector.tensor_tensor(
                            onorm[h][tch][:], o[0:64, :], rs[:], ALU.mult
                        )

        # ---- out projection ---------------------------------------------
        with tc.tile_pool(name="fpsum", bufs=2, space="PSUM") as fpsum, \
                tc.tile_pool(name="ftile", bufs=2) as ftile:
            for tch in range(N_TCH):
                ts = slice(512 * tch, 512 * (tch + 1))
                for m in range(4):
                    po = fpsum.tile([128, 512], f32, tag="po")
                    for h in range(H):
                        nc.tensor.matmul(
                            po[:],
                            r(wot_sb[h][:, 128 * m : 128 * (m + 1)]),
                            r(onorm[h][tch][:]),
                            start=(h == 0), stop=(h == H - 1),
                        )
                    ob = ftile.tile([128, 512], f32, tag="ob")
                    # add bo (per-partition scalar broadcast along t)
                    nc.vector.tensor_scalar_add(ob[:], po[:], bo_sb[:, m : m + 1])
                    nc.sync.dma_start(out_re[:, m, ts], ob[:])

    nc.finalize()
    return nc


# ---------------------------------------------------------------------------
# Host-side input prep per core
# ---------------------------------------------------------------------------


def _head_swap_perm():
    a = np.arange(ATT)
    h = a // HD
    j = a % HD
    return h * HD + (j + 32) % HD


def _rope_tables(pos, length, n):
    theta = ROPE_GAMMA / 10000.0 ** (np.arange(0, HD, 2, dtype=np.float64) / HD)
    f = pos[None, :].astype(np.float64) / max(float(length), 1e-30) * theta[:, None]
    c32 = np.cos(f).astype(np.float32)  # [32, n]
    s32 = np.sin(f).astype(np.float32)
    chalf = np.concatenate([c32, c32], axis=0)  # [64, n]
    shalf = np.concatenate([-s32, s32], axis=0)
    ctab = np.concatenate([chalf, chalf], axis=0)  # [128, n] (2 heads)
    stab = np.concatenate([shalf, shalf], axis=0)
    return np.ascontiguousarray(ctab), np.ascontiguousarray(stab)


def _prep_core_inputs(core, x, context, x_mask, context_mask,
                      Wq, bq, Wk, bk, Wv, bv, Wo, bo, cfg):
    b = core // 2
    th = core % 2
    t0 = th * T_CORE
    perm = _head_swap_perm()

    wqt = np.ascontiguousarray(Wq.T)
    wkt = np.ascontiguousarray(Wk.T)
    m = {
        "x": np.ascontiguousarray(x[b][:, t0 : t0 + T_CORE]),
        "ctxT": np.ascontiguousarray(context[b].T),
        "wqt": wqt,
        "wqts": np.ascontiguousarray(wqt[:, perm]),
        "wkt": wkt,
        "wkts": np.ascontiguousarray(wkt[:, perm]),
        "wvt": np.ascontiguousarray(Wv.T),
        "wot": np.ascontiguousarray(Wo.T),
        "bo": np.ascontiguousarray(bo),
    }
    len_q = float(x_mask[b].sum())
    len_k = float(context_mask[b].sum())
    cq, sq = _rope_tables(np.arange(t0, t0 + T_CORE), len_q, T_CORE)
    ck, sk = _rope_tables(np.arange(L), len_k, L)
    m["cq"], m["sq"], m["ck"], m["sk"] = cq, sq, ck, sk

    if cfg["qk_bias"]:
        # per-partition bias values: columns [bq m-tiles | bq_swapped m-tiles]
        bqv = np.zeros((128, 8), np.float32)
        bkv = np.zeros((128, 8), np.float32)
        for mt in range(4):
            bqv[:, mt] = bq[128 * mt : 128 * (mt + 1)]
            bqv[:, 4 + mt] = bq[perm][128 * mt : 128 * (mt + 1)]
            bkv[:, mt] = bk[128 * mt : 128 * (mt + 1)]
            bkv[:, 4 + mt] = bk[perm][128 * mt : 128 * (mt + 1)]
        m["bqv"], m["bkv"] = bqv, bkv
    if cfg["v_bias"]:
        m["bvt"] = np.ascontiguousarray(
            np.broadcast_to(bv[None, :], (128, ATT)).astype(np.float32)
        )
    if cfg["kmask"]:
        cm = context_mask[b].reshape(L)
        kmb = np.zeros((128, N_LT), np.float32)
        for lt in range(N_LT):
            kmb[:, lt] = np.where(cm[128 * lt : 128 * (lt + 1)] == 0, MASK_NEG, 0.0)
        m["kmb"] = kmb
    return m


def core_slices(c):
    """Index into the full [B, D_MODEL, T] output owned by core c."""
    b, th = c // 2, c % 2
    return (b, slice(None), slice(th * T_CORE, (th + 1) * T_CORE))


def kernel(**inputs):
    from concourse.bass_utils import run_bass_kernel_spmd

    x = np.asarray(inputs["x"], np.float32)
    context = np.asarray(inputs["context"], np.float32)
    x_mask = np.asarray(inputs["x_mask"], np.float32)
    context_mask = np.asarray(inputs["context_mask"], np.float32)
    args = dict(
        x=x, context=context, x_mask=x_mask, context_mask=context_mask,
        Wq=np.asarray(inputs["Wq"], np.float32),
        bq=np.asarray(inputs["bq"], np.float32),
        Wk=np.asarray(inputs["Wk"], np.float32),
        bk=np.asarray(inputs["bk"], np.float32),
        Wv=np.asarray(inputs["Wv"], np.float32),
        bv=np.asarray(inputs["bv"], np.float32),
        Wo=np.asarray(inputs["Wo"], np.float32),
        bo=np.asarray(inputs["bo"], np.float32),
    )

    cfg = {
        "qk_bias": bool(np.any(args["bq"]) or np.any(args["bk"])),
        "v_bias": bool(np.any(args["bv"])),
        "kmask": bool(np.any(context_mask == 0)),
    }

    nc = _build_nc(cfg)
    in_maps = [_prep_core_inputs(c, cfg=cfg, **args) for c in range(N_CORES)]
    res = run_bass_kernel_spmd(nc, in_maps, list(range(N_CORES)))

    out = np.empty((B, D_MODEL, T), np.float32)
    for c in range(N_CORES):
        b, th = c // 2, c % 2
        out[b][:, th * T_CORE : (th + 1) * T_CORE] = res.results[c]["out"]
    # x_mask gate (exact; all-ones in this problem)
    out *= x_mask  # [B,1,T] broadcasts over D_MODEL
    return out



# revision 12
# speedup vs baseline: 1.0934x; 1.0934x over previous
# Trainium2 Bass kernel for nn_AttentionModule_16011638080155.
#
# Cross-attention with length-normalized RoPE, softmax over context L,
# out-projection, written as [B, D_MODEL, T].
#
# Sharding: 8 cores = (batch b in 0..4) x (T half in 0..2). Each core computes
# its full attention output slice [D_MODEL, 1024] independently (k/v projection
# duplicated across the two T-halves of a batch; no collectives).
#
# Device layout (per core), everything "S-transposed" so softmax output feeds
# the PV matmul with no transposes:
#   q.T  [a=512, t=1024]  = WqT.T @ x        (fp8 DoubleRow, weights x256)
#   k.T  [a=512, l=2048]  = WkT.T @ ctxT    (fp8 DoubleRow, weights x256)
#   v    [l=2048, a=512]  = ctxT.T @ WvT    (bf16; stored per l-tile, ones-aug)
#   S.T  [l, t]           = k_rope.T x q_rope (f16, per head, row-tiled pairs)
#   P.T  = exp(S.T / sqrt(512))              (ACT, per-partition mask bias)
#   O    [65, t] = [V | 1s].T @ P.T          (row 64 = softmax sums)
#   out  [dm, t] = WoT.T @ (O * recip(sums)) (+bo)
# The 1/256 weight descale is folded into the f16 rope cos/sin tables.
import math

import numpy as np

# ---------------------------------------------------------------------------
# Workaround for walrus CoreV2/V3 "Too many sync wait commands" on the Tile
# kernel-tail drain: move the accumulated sem waits off the single Drain
# instruction onto preceding nop instructions (same engine, in-order), at
# most 1 wait per instruction.
# ---------------------------------------------------------------------------


def _install_tile_drain_patch():
    import concourse.mybir as mybir
    import concourse.tile as tile_mod
    from concourse.vector_clock import ScopedClock

    if getattr(tile_mod.TileContext, "_drain_patch_installed", False):
        return

    def _patched_drain_and_barrier(self, tick_clock, wait_clock):
        nc = self.nc
        sink = nc.sync.nop(nofuse=True)
        wait_clock.add_sem_waits(
            sink.ins, ScopedClock({None: tick_clock.global_clock})
        )
        si = sink.ins.sync_info
        waits = list(si.on_wait) if si is not None else []
        if len(waits) > 1:
            sink.ins.sync_info = mybir.SyncInfo(on_wait=waits[:1], on_update=[])
            rest = waits[1:]
            for i in range(len(rest)):
                n2 = nc.sync.nop(nofuse=True)
                n2.ins.sync_info = mybir.SyncInfo(
                    on_wait=rest[i : i + 1], on_update=[]
                )
        nc.sync.drain()

        nc.all_engine_barrier()
        assert self.sems is not None
        popped = nc._tile_sem_poison_stack.pop()
        assert popped is self._sem_poison
        nc.clear_and_free_semaphores(list(self.sems.allocated().values()))
        nc.all_engine_barrier()

    tile_mod.TileContext._drain_and_barrier = _patched_drain_and_barrier
    tile_mod.TileContext._drain_patch_installed = True


# ---------------------------------------------------------------------------
# Problem constants (hardcoded per the harness contract).
# ---------------------------------------------------------------------------
B = 4
D_MODEL = 512
T = 2048
L = 2048
D_CTX = 512
ATT = 512
H = 8
HD = 64
ROPE_GAMMA = 10.0
SCALE = math.sqrt(ATT)

N_CORES = 8
T_CORE = T // 2  # 1024, each core handles half the query positions
N_TCH = T_CORE // 512  # 2 chunks of 512
N_LCH = L // 512  # 4
N_LT = L // 128  # 16
MASK_NEG = -60.0  # applied post-scale inside exp(); exp(-60) ~ 8.8e-27
W8_SCALE = 256.0  # fp8 weight scale; descale folded into rope tables


def _build_nc(cfg):
    """Build the single-core Bass program (same program runs SPMD on 8 cores)."""
    import concourse.bacc as bacc
    import concourse.mybir as mybir
    import concourse.tile as tile
    from contextlib import ExitStack

    _install_tile_drain_patch()

    f32 = mybir.dt.float32
    f16 = mybir.dt.float16
    bf16 = mybir.dt.bfloat16
    f8 = mybir.dt.float8e4
    AF = mybir.ActivationFunctionType
    ALU = mybir.AluOpType
    DR = mybir.MatmulPerfMode.DoubleRow

    def r(ap):
        return ap

    nc = bacc.Bacc("TRN2", target_bir_lowering=False, debug=False)

    # ---- DRAM parameters -------------------------------------------------
    x8 = nc.declare_dram_parameter("x8", [D_MODEL, T_CORE], f8, isOutput=False)
    ctx8 = nc.declare_dram_parameter("ctx8", [D_CTX, L], f8, isOutput=False)
    ctxb = nc.declare_dram_parameter("ctxb", [D_CTX, L], bf16, isOutput=False)
    wqt = nc.declare_dram_parameter("wqt", [D_MODEL, ATT], f8, isOutput=False)
    wqts = nc.declare_dram_parameter("wqts", [D_MODEL, ATT], f8, isOutput=False)
    wkt = nc.declare_dram_parameter("wkt", [D_CTX, ATT], f8, isOutput=False)
    wkts = nc.declare_dram_parameter("wkts", [D_CTX, ATT], f8, isOutput=False)
    wvt = nc.declare_dram_parameter("wvt", [D_CTX, ATT], bf16, isOutput=False)
    wot = nc.declare_dram_parameter("wot", [ATT, D_MODEL], f16, isOutput=False)
    cq = nc.declare_dram_parameter("cq", [128, T_CORE], f16, isOutput=False)
    sq = nc.declare_dram_parameter("sq", [128, T_CORE], f16, isOutput=False)
    ck = nc.declare_dram_parameter("ck", [128, L], f16, isOutput=False)
    sk = nc.declare_dram_parameter("sk", [128, L], f16, isOutput=False)
    bo = nc.declare_dram_parameter("bo", [D_MODEL], f32, isOutput=False)
    # broadcast selector: sel[h, 128*hp + c] = 1 iff h == 2*hp + (c >= 64)
    sel = nc.declare_dram_parameter("sel", [8, 512], f16, isOutput=False)
    if cfg["qk_bias"]:
        # per-partition bias values: columns [bq(4 m-tiles) | swapped(4)]
        bqv = nc.declare_dram_parameter("bqv", [128, 8], f32, isOutput=False)
        bkv = nc.declare_dram_parameter("bkv", [128, 8], f32, isOutput=False)
    if cfg["v_bias"]:
        bvt = nc.declare_dram_parameter("bvt", [128, ATT], f32, isOutput=False)
    if cfg["kmask"]:
        kmb = nc.declare_dram_parameter("kmb", [128, N_LT], f32, isOutput=False)
    out = nc.declare_dram_parameter("out", [D_MODEL, T_CORE], f32, isOutput=True)

    x8_re = x8.rearrange("(kp p) t -> p kp t", p=128)
    ctx8_re = ctx8.rearrange("(kp p) l -> p kp l", p=128)
    ctxb_re = ctxb.rearrange("(kp p) l -> p kp l", p=128)
    wqt_re = wqt.rearrange("(kp p) a -> p kp a", p=128)
    wqts_re = wqts.rearrange("(kp p) a -> p kp a", p=128)
    wkt_re = wkt.rearrange("(kp p) a -> p kp a", p=128)
    wkts_re = wkts.rearrange("(kp p) a -> p kp a", p=128)
    wvt_re = wvt.rearrange("(kp p) a -> p kp a", p=128)
    bo_re = bo.rearrange("(kp p) -> p kp", p=128)
    out_re = out.rearrange("(kp p) t -> p kp t", p=128)

    with tile.TileContext(nc) as tc, ExitStack() as ctx:
        # ---- persistent SBUF tiles --------------------------------------
        per = ctx.enter_context(tc.tile_pool(name="per", bufs=1))
        qropeT = [per.tile([128, T_CORE], f16, tag=f"qrope{m}", name=f"qrope{m}") for m in range(4)]
        kropeT = [per.tile([128, L], f16, tag=f"krope{m}", name=f"krope{m}") for m in range(4)]
        vaug = [per.tile([128, H * 65], f16, tag=f"vaug{lt}", name=f"vaug{lt}") for lt in range(N_LT)]
        # unnormalized PV outputs, head pairs stacked on partitions
        oraw = [
            [per.tile([128, 512], f16, tag=f"or{tch}_{hp}", name=f"or{tch}_{hp}") for hp in range(4)]
            for tch in range(N_TCH)
        ]
        onorm = [
            [per.tile([128, 512], f16, tag=f"on{tch}_{hp}", name=f"on{tch}_{hp}") for hp in range(4)]
            for tch in range(N_TCH)
        ]
        # softmax sums per tch, rows = head index (filled via DMA)
        sums_sb = [
            per.tile([8, 512], f32, tag=f"sums{tch}", name=f"sums{tch}")
            for tch in range(N_TCH)
        ]
        sel_sb = per.tile([8, 512], f16, tag="sel")
        nc.sync.dma_start(sel_sb[:], sel[:])
        wot_sb = [per.tile([128, D_MODEL], f16, tag=f"wot{hp}", name=f"wot{hp}") for hp in range(4)]
        bo_sb = per.tile([128, 4], f32, tag="bo")
        if cfg["kmask"]:
            kmb_sb = per.tile([128, N_LT], f32, tag="kmb")
            nc.sync.dma_start(kmb_sb[:], kmb[:])

        for hp in range(4):
            nc.sync.dma_start(wot_sb[hp][:], wot[128 * hp : 128 * (hp + 1), :])
        nc.sync.dma_start(bo_sb[:], bo_re)

        # ---- phase Q: q.T projection + rope (fp8 DoubleRow) -------------
        with tc.tile_pool(name="qph", bufs=1) as qph, tc.tile_pool(
            name="qpsum", bufs=2, space="PSUM"
        ) as qpsum, tc.tile_pool(name="qtmp", bufs=4) as qtmp:
            x_sb = qph.tile([128, 4, T_CORE], f8, tag="x")
            wq_sb = qph.tile([128, 4, ATT], f8, tag="wq")
            wqs_sb = qph.tile([128, 4, ATT], f8, tag="wqs")
            cq_sb = qph.tile([128, T_CORE], f16, tag="cq")
            sq_sb = qph.tile([128, T_CORE], f16, tag="sq")
            nc.sync.dma_start(x_sb[:], x8_re)
            nc.sync.dma_start(wq_sb[:], wqt_re)
            nc.sync.dma_start(wqs_sb[:], wqts_re)
            nc.sync.dma_start(cq_sb[:], cq[:])
            nc.sync.dma_start(sq_sb[:], sq[:])
            if cfg["qk_bias"]:
                bq_sb = qph.tile([128, 8], f32, tag="bq")
                nc.sync.dma_start(bq_sb[:], bqv[:])

            for m in range(4):
                for tch in range(N_TCH):
                    ts = slice(512 * tch, 512 * (tch + 1))
                    pc = qpsum.tile([128, 512], f32, tag="pc")
                    ps = qpsum.tile([128, 512], f32, tag="ps")
                    for i in range(2):
                        nc.tensor.matmul(
                            pc[:],
                            r(wq_sb[:, 2 * i : 2 * i + 2, 128 * m : 128 * (m + 1)]),
                            r(x_sb[:, 2 * i : 2 * i + 2, ts]),
                            start=(i == 0), stop=(i == 1), perf_mode=DR,
                        )
                        nc.tensor.matmul(
                            ps[:],
                            r(wqs_sb[:, 2 * i : 2 * i + 2, 128 * m : 128 * (m + 1)]),
                            r(x_sb[:, 2 * i : 2 * i + 2, ts]),
                            start=(i == 0), stop=(i == 1), perf_mode=DR,
                        )
                    if cfg["qk_bias"]:
                        # bias (pre-rope) per-partition; W8_SCALE folded out
                        # of psum via the tables, so add bias*W8_SCALE here.
                        nc.vector.tensor_scalar_add(
                            pc[:], pc[:], bq_sb[:, m : m + 1]
                        )
                        nc.vector.tensor_scalar_add(
                            ps[:], ps[:], bq_sb[:, 4 + m : 5 + m]
                        )
                    t1 = qtmp.tile([128, 512], f16, tag="t1")
                    t2 = qtmp.tile([128, 512], f16, tag="t2")
                    nc.vector.tensor_tensor(t1[:], pc[:], cq_sb[:, ts], ALU.mult)
                    nc.vector.tensor_tensor(t2[:], ps[:], sq_sb[:, ts], ALU.mult)
                    nc.vector.tensor_tensor(
                        qropeT[m][:, ts], t1[:], t2[:], ALU.add
                    )

        # ---- phase KV: k.T projection + rope (fp8 DR), v proj (bf16) ----
        with tc.tile_pool(name="kph", bufs=1) as kph, tc.tile_pool(
            name="kpsum", bufs=2, space="PSUM"
        ) as kpsum, tc.tile_pool(name="ktmp", bufs=4) as ktmp:
            wk_sb = kph.tile([128, 4, ATT], f8, tag="wk")
            wks_sb = kph.tile([128, 4, ATT], f8, tag="wks")
            wv_sb = kph.tile([128, 4, ATT], bf16, tag="wv")
            ctx8_sb = kph.tile([128, 4, L], f8, tag="ctx8")
            ctxb_sb = kph.tile([128, 4, L], bf16, tag="ctxb")
            ck_sb = kph.tile([128, L], f16, tag="ck")
            sk_sb = kph.tile([128, L], f16, tag="sk")
            nc.sync.dma_start(wk_sb[:], wkt_re)
            nc.sync.dma_start(wks_sb[:], wkts_re)
            nc.sync.dma_start(wv_sb[:], wvt_re)
            nc.sync.dma_start(ctx8_sb[:], ctx8_re)
            nc.sync.dma_start(ctxb_sb[:], ctxb_re)
            nc.sync.dma_start(ck_sb[:], ck[:])
            nc.sync.dma_start(sk_sb[:], sk[:])
            if cfg["qk_bias"]:
                bk_sb = kph.tile([128, 8], f32, tag="bk")
                nc.sync.dma_start(bk_sb[:], bkv[:])
            if cfg["v_bias"]:
                bv_sb = kph.tile([128, ATT], f32, tag="bv")
                nc.sync.dma_start(bv_sb[:], bvt[:])

            for lch in range(N_LCH):
                ls = slice(512 * lch, 512 * (lch + 1))
                # k.T for this l chunk: all 4 a-tiles
                for m in range(4):
                    pc = kpsum.tile([128, 512], f32, tag="kc")
                    ps = kpsum.tile([128, 512], f32, tag="ks")
                    for i in range(2):
                        nc.tensor.matmul(
                            pc[:],
                            r(wk_sb[:, 2 * i : 2 * i + 2, 128 * m : 128 * (m + 1)]),
                            r(ctx8_sb[:, 2 * i : 2 * i + 2, ls]),
                            start=(i == 0), stop=(i == 1), perf_mode=DR,
                        )
                        nc.tensor.matmul(
                            ps[:],
                            r(wks_sb[:, 2 * i : 2 * i + 2, 128 * m : 128 * (m + 1)]),
                            r(ctx8_sb[:, 2 * i : 2 * i + 2, ls]),
                            start=(i == 0), stop=(i == 1), perf_mode=DR,
                        )
                    if cfg["qk_bias"]:
                        nc.vector.tensor_scalar_add(
                            pc[:], pc[:], bk_sb[:, m : m + 1]
                        )
                        nc.vector.tensor_scalar_add(
                            ps[:], ps[:], bk_sb[:, 4 + m : 5 + m]
                        )
                    t1 = ktmp.tile([128, 512], f16, tag="kt1")
                    t2 = ktmp.tile([128, 512], f16, tag="kt2")
                    nc.vector.tensor_tensor(t1[:], pc[:], ck_sb[:, ls], ALU.mult)
                    nc.vector.tensor_tensor(t2[:], ps[:], sk_sb[:, ls], ALU.mult)
                    nc.vector.tensor_tensor(
                        kropeT[m][:, ls], t1[:], t2[:], ALU.add
                    )

                # v for the 4 l-tiles of this chunk (bf16 inputs)
                for j in range(4):
                    lt = 4 * lch + j
                    pv = kpsum.tile([128, 512], f32, tag="vps")
                    for k in range(4):
                        nc.tensor.matmul(
                            pv[:],
                            r(ctxb_sb[:, k, 128 * lt : 128 * (lt + 1)]),
                            r(wv_sb[:, k, :]),
                            start=(k == 0),
                            stop=(k == 3),
                        )
                    if cfg["v_bias"]:
                        nc.vector.tensor_tensor(pv[:], pv[:], bv_sb[:], ALU.add)
                    va = vaug[lt][:].rearrange("p (h e) -> p h e", e=65)
                    nc.vector.tensor_copy(
                        va[:, :, 0:64],
                        pv[:].rearrange("p (h d) -> p h d", h=H),
                    )
                    nc.vector.memset(va[:, :, 64], 1.0)

        # ---- attention ---------------------------------------------------
        with tc.tile_pool(name="apsum", bufs=2, space="PSUM") as apsum, \
                tc.tile_pool(name="opsum", bufs=2, space="PSUM") as opsum, \
                tc.tile_pool(name="ptile", bufs=4) as ptile, \
                tc.tile_pool(name="stile", bufs=4) as stile:
            for tch in range(N_TCH):
                ts = slice(512 * tch, 512 * (tch + 1))
                for hp in range(4):
                    hA, hB = 2 * hp, 2 * hp + 1
                    oA = opsum.tile([65, 512], f32, tag="oA")
                    oB = opsum.tile([65, 512], f32, tag="oB")
                    for lt in range(N_LT):
                        lw = slice(128 * lt, 128 * (lt + 1))
                        g = apsum.tile([128, 1024], f32, tag="g")
                        nc.tensor.matmul(
                            g[:, 0:512],
                            r(kropeT[hp][0:64, lw]),
                            r(qropeT[hp][0:64, ts]),
                            start=True, stop=True,
                            tile_position=(0, 0),
                        )
                        nc.tensor.matmul(
                            g[:, 512:1024],
                            r(kropeT[hp][64:128, lw]),
                            r(qropeT[hp][64:128, ts]),
                            start=True, stop=True,
                            tile_position=(64, 0),
                        )
                        pT = ptile.tile([128, 1024], f16, tag="pT")
                        if cfg["kmask"]:
                            nc.scalar.activation(
                                pT[:], g[:], AF.Exp,
                                bias=kmb_sb[:, lt : lt + 1],
                                scale=1.0 / SCALE,
                            )
                        else:
                            nc.scalar.activation(
                                pT[:], g[:], AF.Exp, scale=1.0 / SCALE
                            )
                        nc.tensor.matmul(
                            oA[:],
                            r(vaug[lt][:, 65 * hA : 65 * hA + 65]),
                            r(pT[:, 0:512]),
                            start=(lt == 0), stop=(lt == N_LT - 1),
                        )
                        nc.tensor.matmul(
                            oB[:],
                            r(vaug[lt][:, 65 * hB : 65 * hB + 65]),
                            r(pT[:, 512:1024]),
                            start=(lt == 0), stop=(lt == N_LT - 1),
                        )
                    # evacuate unnormalized O; sums rows go to SBUF via DMA
                    # (engine copies cannot target unaligned partitions)
                    nc.vector.tensor_copy(
                        oraw[tch][hp][0:64, :], oA[0:64, :]
                    )
                    nc.vector.tensor_copy(
                        oraw[tch][hp][64:128, :], oB[0:64, :]
                    )
                    # stage the sums rows at partition 0, then DMA to row h
                    # (engine copies cannot target unaligned partitions)
                    for h, o in ((hA, oA), (hB, oB)):
                        srow = stile.tile([1, 512], f32, tag="srow")
                        nc.vector.tensor_copy(srow[:], o[64:65, :])
                        nc.sync.dma_start(sums_sb[tch][h : h + 1, :], srow[:])

        # ---- normalize + out projection ---------------------------------
        with tc.tile_pool(name="fpsum", bufs=2, space="PSUM") as fpsum, \
                tc.tile_pool(name="npsum", bufs=2, space="PSUM") as npsum, \
                tc.tile_pool(name="ftile", bufs=4) as ftile:
            for tch in range(N_TCH):
                ts = slice(512 * tch, 512 * (tch + 1))
                # batched softmax normalization: one reciprocal for 8 heads,
                # broadcast to head-pair partitions via a tiny select matmul
                rinv = ftile.tile([8, 512], f32, tag="rinv")
                nc.vector.reciprocal(rinv[:], sums_sb[tch][:])
                rinvh = ftile.tile([8, 512], f16, tag="rinvh")
                nc.vector.tensor_copy(rinvh[:], rinv[:])
                for hp in range(4):
                    rs = npsum.tile([128, 512], f32, tag="rs")
                    nc.tensor.matmul(
                        rs[:],
                        r(sel_sb[:, 128 * hp : 128 * (hp + 1)]),
                        r(rinvh[:]),
                        start=True, stop=True,
                    )
                    nc.vector.tensor_tensor(
                        onorm[tch][hp][:], oraw[tch][hp][:], rs[:], ALU.mult
                    )
                for m in range(4):
                    po = fpsum.tile([128, 512], f32, tag="po")
                    for hp in range(4):
                        nc.tensor.matmul(
                            po[:],
                            r(wot_sb[hp][:, 128 * m : 128 * (m + 1)]),
                            r(onorm[tch][hp][:]),
                            start=(hp == 0), stop=(hp == 3),
                        )
                    ob = ftile.tile([128, 512], f32, tag="ob")
                    # add bo (per-partition scalar broadcast along t)
                    nc.vector.tensor_scalar_add(ob[:], po[:], bo_sb[:, m : m + 1])
                    nc.sync.dma_start(out_re[:, m, ts], ob[:])

    nc.finalize()
    return nc


# ---------------------------------------------------------------------------
# Host-side input prep per core
# ---------------------------------------------------------------------------


def _head_swap_perm():
    a = np.arange(ATT)
    h = a // HD
    j = a % HD
    return h * HD + (j + 32) % HD


def _rope_tables(pos, length, n, scale):
    theta = ROPE_GAMMA / 10000.0 ** (np.arange(0, HD, 2, dtype=np.float64) / HD)
    f = pos[None, :].astype(np.float64) / max(float(length), 1e-30) * theta[:, None]
    c32 = (np.cos(f) * scale).astype(np.float32)  # [32, n]
    s32 = (np.sin(f) * scale).astype(np.float32)
    chalf = np.concatenate([c32, c32], axis=0)  # [64, n]
    shalf = np.concatenate([-s32, s32], axis=0)
    ctab = np.concatenate([chalf, chalf], axis=0)  # [128, n] (2 heads)
    stab = np.concatenate([shalf, shalf], axis=0)
    return np.ascontiguousarray(ctab), np.ascontiguousarray(stab)


def _to_f8(a):
    import ml_dtypes

    return np.ascontiguousarray(
        np.clip(a, -240.0, 240.0).astype(ml_dtypes.float8_e4m3)
    )


def _to_bf16(a):
    import ml_dtypes

    return np.ascontiguousarray(a.astype(ml_dtypes.bfloat16))


def _to_f16(a):
    return np.ascontiguousarray(a.astype(np.float16))


def _prep_core_inputs(core, x, context, x_mask, context_mask,
                      Wq, bq, Wk, bk, Wv, bv, Wo, bo, cfg):
    b = core // 2
    th = core % 2
    t0 = th * T_CORE
    perm = _head_swap_perm()

    wqt = Wq.T * W8_SCALE
    wkt = Wk.T * W8_SCALE
    ctxT = np.ascontiguousarray(context[b].T)
    m = {
        "x8": _to_f8(x[b][:, t0 : t0 + T_CORE]),
        "ctx8": _to_f8(ctxT),
        "ctxb": _to_bf16(ctxT),
        "wqt": _to_f8(wqt),
        "wqts": _to_f8(wqt[:, perm]),
        "wkt": _to_f8(wkt),
        "wkts": _to_f8(wkt[:, perm]),
        "wvt": _to_bf16(Wv.T),
        "wot": _to_f16(Wo.T),
        "bo": np.ascontiguousarray(bo.astype(np.float32)),
    }
    selm = np.zeros((8, 512), np.float32)
    for hp in range(4):
        selm[2 * hp, 128 * hp : 128 * hp + 64] = 1.0
        selm[2 * hp + 1, 128 * hp + 64 : 128 * hp + 128] = 1.0
    m["sel"] = _to_f16(selm)

    len_q = float(x_mask[b].sum())
    len_k = float(context_mask[b].sum())
    # 1/W8_SCALE descale of the fp8 projection folded into the tables
    cqt, sqt = _rope_tables(
        np.arange(t0, t0 + T_CORE), len_q, T_CORE, 1.0 / W8_SCALE
    )
    ckt, skt = _rope_tables(np.arange(L), len_k, L, 1.0 / W8_SCALE)
    m["cq"], m["sq"] = _to_f16(cqt), _to_f16(sqt)
    m["ck"], m["sk"] = _to_f16(ckt), _to_f16(skt)

    if cfg["qk_bias"]:
        # per-partition bias values, pre-scaled to match the fp8 psum
        # (psum holds W8_SCALE * (Wx); tables divide by W8_SCALE, so the
        # bias added in psum must also be scaled by W8_SCALE)
        bqv = np.zeros((128, 8), np.float32)
        bkv = np.zeros((128, 8), np.float32)
        for mt in range(4):
            bqv[:, mt] = bq[128 * mt : 128 * (mt + 1)] * W8_SCALE
            bqv[:, 4 + mt] = bq[perm][128 * mt : 128 * (mt + 1)] * W8_SCALE
            bkv[:, mt] = bk[128 * mt : 128 * (mt + 1)] * W8_SCALE
            bkv[:, 4 + mt] = bk[perm][128 * mt : 128 * (mt + 1)] * W8_SCALE
        m["bqv"], m["bkv"] = bqv, bkv
    if cfg["v_bias"]:
        m["bvt"] = np.ascontiguousarray(
            np.broadcast_to(bv[None, :], (128, ATT)).astype(np.float32)
        )
    if cfg["kmask"]:
        cm = context_mask[b].reshape(L)
        kmb = np.zeros((128, N_LT), np.float32)
        for lt in range(N_LT):
            kmb[:, lt] = np.where(cm[128 * lt : 128 * (lt + 1)] == 0, MASK_NEG, 0.0)
        m["kmb"] = kmb
    return m


def core_slices(c):
    """Index into the full [B, D_MODEL, T] output owned by core c."""
    b, th = c // 2, c % 2
    return (b, slice(None), slice(th * T_CORE, (th + 1) * T_CORE))


def kernel(**inputs):
    from concourse.bass_utils import run_bass_kernel_spmd

    x = np.asarray(inputs["x"], np.float32)
    context = np.asarray(inputs["context"], np.float32)
    x_mask = np.asarray(inputs["x_mask"], np.float32)
    context_mask = np.asarray(inputs["context_mask"], np.float32)
    args = dict(
        x=x, context=context, x_mask=x_mask, context_mask=context_mask,
        Wq=np.asarray(inputs["Wq"], np.float32),
        bq=np.asarray(inputs["bq"], np.float32),
        Wk=np.asarray(inputs["Wk"], np.float32),
        bk=np.asarray(inputs["bk"], np.float32),
        Wv=np.asarray(inputs["Wv"], np.float32),
        bv=np.asarray(inputs["bv"], np.float32),
        Wo=np.asarray(inputs["Wo"], np.float32),
        bo=np.asarray(inputs["bo"], np.float32),
    )

    cfg = {
        "qk_bias": bool(np.any(args["bq"]) or np.any(args["bk"])),
        "v_bias": bool(np.any(args["bv"])),
        "kmask": bool(np.any(context_mask == 0)),
    }

    nc = _build_nc(cfg)
    in_maps = [_prep_core_inputs(c, cfg=cfg, **args) for c in range(N_CORES)]
    res = run_bass_kernel_spmd(nc, in_maps, list(range(N_CORES)))

    out = np.empty((B, D_MODEL, T), np.float32)
    for c in range(N_CORES):
        out[core_slices(c)] = res.results[c]["out"]
    # x_mask gate (exact; all-ones in this problem)
    out = out * x_mask  # [B,1,T] broadcasts over D_MODEL
    return out


# revision 26
# speedup vs baseline: 1.1733x; 1.0731x over previous
# Trainium2 Bass kernel for nn_AttentionModule_16011638080155.
#
# Cross-attention with length-normalized RoPE, softmax over context L,
# out-projection, written as [B, D_MODEL, T].
#
# Sharding: 8 cores = (batch b in 0..4) x (T half in 0..2). Each core computes
# its full attention output slice [D_MODEL, 1024] independently (k/v projection
# duplicated across the two T-halves of a batch; no collectives).
#
# Device layout (per core), everything "S-transposed" so softmax output feeds
# the PV matmul with no transposes:
#   q.T  [a=512, t=1024]  = WqT.T @ x        (fp8 DoubleRow, weights x256)
#   k.T  [a=512, l=2048]  = WkT.T @ ctxT    (fp8 DoubleRow, weights x256)
#   v    [l=2048, a=512]  = ctxT.T @ WvT    (fp8 DR; stored per l-tile, ones-aug)
#   S.T  [l, t]           = k_rope.T x q_rope (f16, per head, row-tiled pairs)
#   P.T  = exp(S.T / sqrt(512))              (ACT, per-partition mask bias)
#   O    [65, t] = [V | 1s].T @ P.T          (row 64 = softmax sums)
#   out  [dm, t] = WoT.T @ (O * recip(sums)) (+bo)
# The 1/256 weight descale is folded into the f16 rope cos/sin tables.
import math

import numpy as np

# ---------------------------------------------------------------------------
# Workaround for walrus CoreV2/V3 "Too many sync wait commands" on the Tile
# kernel-tail drain: move the accumulated sem waits off the single Drain
# instruction onto preceding nop instructions (same engine, in-order), at
# most 1 wait per instruction.
# ---------------------------------------------------------------------------


def _install_tile_drain_patch():
    import concourse.mybir as mybir
    import concourse.tile as tile_mod
    from concourse.vector_clock import ScopedClock

    if getattr(tile_mod.TileContext, "_drain_patch_installed", False):
        return

    def _patched_drain_and_barrier(self, tick_clock, wait_clock):
        nc = self.nc
        sink = nc.sync.nop(nofuse=True)
        wait_clock.add_sem_waits(
            sink.ins, ScopedClock({None: tick_clock.global_clock})
        )
        si = sink.ins.sync_info
        waits = list(si.on_wait) if si is not None else []
        if len(waits) > 1:
            sink.ins.sync_info = mybir.SyncInfo(on_wait=waits[:1], on_update=[])
            rest = waits[1:]
            for i in range(len(rest)):
                n2 = nc.sync.nop(nofuse=True)
                n2.ins.sync_info = mybir.SyncInfo(
                    on_wait=rest[i : i + 1], on_update=[]
                )
        nc.sync.drain()

        nc.all_engine_barrier()
        assert self.sems is not None
        popped = nc._tile_sem_poison_stack.pop()
        assert popped is self._sem_poison
        nc.clear_and_free_semaphores(list(self.sems.allocated().values()))
        nc.all_engine_barrier()

    tile_mod.TileContext._drain_and_barrier = _patched_drain_and_barrier
    tile_mod.TileContext._drain_patch_installed = True


# ---------------------------------------------------------------------------
# Problem constants (hardcoded per the harness contract).
# ---------------------------------------------------------------------------
B = 4
D_MODEL = 512
T = 2048
L = 2048
D_CTX = 512
ATT = 512
H = 8
HD = 64
ROPE_GAMMA = 10.0
SCALE = math.sqrt(ATT)

N_CORES = 8
T_CORE = T // 2  # 1024, each core handles half the query positions
N_TCH = T_CORE // 512  # 2 chunks of 512
N_LCH = L // 512  # 4
N_LT = L // 128  # 16
MASK_NEG = -60.0  # applied post-scale inside exp(); exp(-60) ~ 8.8e-27
W8_SCALE = 256.0  # fp8 weight scale; descale folded into rope tables


def _build_nc(cfg):
    """Build the single-core Bass program (same program runs SPMD on 8 cores)."""
    import concourse.bacc as bacc
    import concourse.mybir as mybir
    import concourse.tile as tile
    from contextlib import ExitStack

    _install_tile_drain_patch()

    f32 = mybir.dt.float32
    f16 = mybir.dt.float16
    bf16 = mybir.dt.bfloat16
    f8 = mybir.dt.float8e4
    AF = mybir.ActivationFunctionType
    ALU = mybir.AluOpType
    DR = mybir.MatmulPerfMode.DoubleRow

    def r(ap):
        return ap

    nc = bacc.Bacc("TRN2", target_bir_lowering=False, debug=False)

    # ---- DRAM parameters -------------------------------------------------
    x8 = nc.declare_dram_parameter("x8", [D_MODEL, T_CORE], f8, isOutput=False)
    ctx8 = nc.declare_dram_parameter("ctx8", [D_CTX, L], f8, isOutput=False)
    wqt = nc.declare_dram_parameter("wqt", [D_MODEL, ATT], f8, isOutput=False)
    wqts = nc.declare_dram_parameter("wqts", [D_MODEL, ATT], f8, isOutput=False)
    wkt = nc.declare_dram_parameter("wkt", [D_CTX, ATT], f8, isOutput=False)
    wkts = nc.declare_dram_parameter("wkts", [D_CTX, ATT], f8, isOutput=False)
    ctxb = nc.declare_dram_parameter("ctxb", [D_CTX, L], bf16, isOutput=False)
    wvt = nc.declare_dram_parameter("wvt", [D_CTX, ATT], bf16, isOutput=False)
    wot = nc.declare_dram_parameter("wot", [ATT, D_MODEL], f16, isOutput=False)
    cq = nc.declare_dram_parameter("cq", [128, T_CORE], f16, isOutput=False)
    sq = nc.declare_dram_parameter("sq", [128, T_CORE], f16, isOutput=False)
    ck = nc.declare_dram_parameter("ck", [128, L], f16, isOutput=False)
    sk = nc.declare_dram_parameter("sk", [128, L], f16, isOutput=False)
    bo = nc.declare_dram_parameter("bo", [D_MODEL], f32, isOutput=False)
    # broadcast selector: sel[h, 128*hp + c] = 1 iff h == 2*hp + (c >= 64)
    sel = nc.declare_dram_parameter("sel", [8, 512], f16, isOutput=False)
    if cfg["qk_bias"]:
        # per-partition bias values: columns [bq(4 m-tiles) | swapped(4)]
        bqv = nc.declare_dram_parameter("bqv", [128, 8], f32, isOutput=False)
        bkv = nc.declare_dram_parameter("bkv", [128, 8], f32, isOutput=False)
    if cfg["v_bias"]:
        bvt = nc.declare_dram_parameter("bvt", [128, ATT], f32, isOutput=False)
    if cfg["kmask"]:
        kmb = nc.declare_dram_parameter("kmb", [128, N_LT], f32, isOutput=False)
    out = nc.declare_dram_parameter("out", [D_MODEL, T_CORE], f32, isOutput=True)

    x8_re = x8.rearrange("(kp p) t -> p kp t", p=128)
    ctx8_re = ctx8.rearrange("(kp p) l -> p kp l", p=128)
    ctxb_re = ctxb.rearrange("(kp p) l -> p kp l", p=128)
    wqt_re = wqt.rearrange("(kp p) a -> p kp a", p=128)
    wqts_re = wqts.rearrange("(kp p) a -> p kp a", p=128)
    wkt_re = wkt.rearrange("(kp p) a -> p kp a", p=128)
    wkts_re = wkts.rearrange("(kp p) a -> p kp a", p=128)
    wvt_re = wvt.rearrange("(kp p) a -> p kp a", p=128)
    bo_re = bo.rearrange("(kp p) -> p kp", p=128)
    out_re = out.rearrange("(kp p) t -> p kp t", p=128)

    with tile.TileContext(nc) as tc, ExitStack() as ctx:
        # ---- persistent SBUF tiles --------------------------------------
        per = ctx.enter_context(tc.tile_pool(name="per", bufs=1))
        qropeT = [per.tile([128, T_CORE], f16, tag=f"qrope{m}", name=f"qrope{m}") for m in range(4)]
        kropeT = [per.tile([128, L], f16, tag=f"krope{m}", name=f"krope{m}") for m in range(4)]
        vaug = [per.tile([128, H * 65], f16, tag=f"vaug{lt}", name=f"vaug{lt}") for lt in range(N_LT)]
        # unnormalized PV outputs, head pairs stacked on partitions
        oraw = [
            [per.tile([128, 512], f16, tag=f"or{tch}_{hp}", name=f"or{tch}_{hp}") for hp in range(4)]
            for tch in range(N_TCH)
        ]
        onorm = [
            [per.tile([128, 512], f16, tag=f"on{tch}_{hp}", name=f"on{tch}_{hp}") for hp in range(4)]
            for tch in range(N_TCH)
        ]
        # softmax sums per tch, rows = head index (filled via DMA)
        sums_sb = [
            per.tile([8, 512], f32, tag=f"sums{tch}", name=f"sums{tch}")
            for tch in range(N_TCH)
        ]
        sel_sb = per.tile([8, 512], f16, tag="sel")
        wot_sb = [per.tile([128, D_MODEL], f16, tag=f"wot{hp}", name=f"wot{hp}") for hp in range(4)]
        bo_sb = per.tile([128, 4], f32, tag="bo")
        if cfg["kmask"]:
            kmb_sb = per.tile([128, N_LT], f32, tag="kmb")

        # input staging tiles (persistent so all loads prefetch up front)
        x_sb = per.tile([128, 4, T_CORE], f8, tag="x")
        wq_sb = per.tile([128, 4, ATT], f8, tag="wq")
        wqs_sb = per.tile([128, 4, ATT], f8, tag="wqs")
        cq_sb = per.tile([128, T_CORE], f16, tag="cq")
        sq_sb = per.tile([128, T_CORE], f16, tag="sq")
        wk_sb = per.tile([128, 4, ATT], f8, tag="wk")
        wks_sb = per.tile([128, 4, ATT], f8, tag="wks")
        wv_sb = per.tile([128, 4, ATT], bf16, tag="wv")
        ctx8_sb = per.tile([128, 4, L], f8, tag="ctx8")
        ctxb_sb = per.tile([128, 4, L], bf16, tag="ctxb")
        ck_sb = per.tile([128, L], f16, tag="ck")
        sk_sb = per.tile([128, L], f16, tag="sk")

        # prefetch in consumption order: Q phase, then K, then V, then tail
        nc.sync.dma_start(x_sb[:], x8_re)
        nc.sync.dma_start(wq_sb[:], wqt_re)
        nc.sync.dma_start(wqs_sb[:], wqts_re)
        nc.sync.dma_start(cq_sb[:], cq[:])
        nc.sync.dma_start(sq_sb[:], sq[:])
        nc.sync.dma_start(ctx8_sb[:], ctx8_re)
        nc.sync.dma_start(wk_sb[:], wkt_re)
        nc.sync.dma_start(wks_sb[:], wkts_re)
        nc.sync.dma_start(ck_sb[:], ck[:])
        nc.sync.dma_start(sk_sb[:], sk[:])
        nc.sync.dma_start(ctxb_sb[:], ctxb_re)
        nc.sync.dma_start(wv_sb[:], wvt_re)
        for hp in range(4):
            nc.sync.dma_start(wot_sb[hp][:], wot[128 * hp : 128 * (hp + 1), :])
        nc.sync.dma_start(bo_sb[:], bo_re)
        nc.sync.dma_start(sel_sb[:], sel[:])
        if cfg["kmask"]:
            nc.sync.dma_start(kmb_sb[:], kmb[:])
        if cfg["qk_bias"]:
            bq_sb = per.tile([128, 8], f32, tag="bq")
            bk_sb = per.tile([128, 8], f32, tag="bk")
            nc.sync.dma_start(bq_sb[:], bqv[:])
            nc.sync.dma_start(bk_sb[:], bkv[:])
        if cfg["v_bias"]:
            bv_sb = per.tile([128, ATT], f32, tag="bv")
            nc.sync.dma_start(bv_sb[:], bvt[:])

        # ---- phase Q: q.T projection + rope (fp8 DoubleRow) -------------
        with tc.tile_pool(name="qpsum", bufs=2, space="PSUM") as qpsum, \
                tc.tile_pool(name="qtmp", bufs=4) as qtmp:
            for m in range(4):
                for tch in range(N_TCH):
                    ts = slice(512 * tch, 512 * (tch + 1))
                    pc = qpsum.tile([128, 512], f32, tag="pc")
                    ps = qpsum.tile([128, 512], f32, tag="ps")
                    for i in range(2):
                        nc.tensor.matmul(
                            pc[:],
                            r(wq_sb[:, 2 * i : 2 * i + 2, 128 * m : 128 * (m + 1)]),
                            r(x_sb[:, 2 * i : 2 * i + 2, ts]),
                            start=(i == 0), stop=(i == 1), perf_mode=DR,
                        )
                        nc.tensor.matmul(
                            ps[:],
                            r(wqs_sb[:, 2 * i : 2 * i + 2, 128 * m : 128 * (m + 1)]),
                            r(x_sb[:, 2 * i : 2 * i + 2, ts]),
                            start=(i == 0), stop=(i == 1), perf_mode=DR,
                        )
                    if cfg["qk_bias"]:
                        # bias (pre-rope) per-partition; W8_SCALE folded out
                        # of psum via the tables, so add bias*W8_SCALE here.
                        nc.vector.tensor_scalar_add(
                            pc[:], pc[:], bq_sb[:, m : m + 1]
                        )
                        nc.vector.tensor_scalar_add(
                            ps[:], ps[:], bq_sb[:, 4 + m : 5 + m]
                        )
                    t1 = qtmp.tile([128, 512], f16, tag="t1")
                    t2 = qtmp.tile([128, 512], f16, tag="t2")
                    nc.vector.tensor_tensor(t1[:], pc[:], cq_sb[:, ts], ALU.mult)
                    nc.vector.tensor_tensor(t2[:], ps[:], sq_sb[:, ts], ALU.mult)
                    nc.vector.tensor_tensor(
                        qropeT[m][:, ts], t1[:], t2[:], ALU.add
                    )

        # ---- phase KV: k.T projection + rope, v proj (both fp8 DR) ------
        with tc.tile_pool(name="kpsum", bufs=2, space="PSUM") as kpsum, \
                tc.tile_pool(name="ktmp", bufs=4) as ktmp:
            for lch in range(N_LCH):
                ls = slice(512 * lch, 512 * (lch + 1))
                # k.T for this l chunk: all 4 a-tiles
                for m in range(4):
                    pc = kpsum.tile([128, 512], f32, tag="kc")
                    ps = kpsum.tile([128, 512], f32, tag="ks")
                    for i in range(2):
                        nc.tensor.matmul(
                            pc[:],
                            r(wk_sb[:, 2 * i : 2 * i + 2, 128 * m : 128 * (m + 1)]),
                            r(ctx8_sb[:, 2 * i : 2 * i + 2, ls]),
                            start=(i == 0), stop=(i == 1), perf_mode=DR,
                        )
                        nc.tensor.matmul(
                            ps[:],
                            r(wks_sb[:, 2 * i : 2 * i + 2, 128 * m : 128 * (m + 1)]),
                            r(ctx8_sb[:, 2 * i : 2 * i + 2, ls]),
                            start=(i == 0), stop=(i == 1), perf_mode=DR,
                        )
                    if cfg["qk_bias"]:
                        nc.vector.tensor_scalar_add(
                            pc[:], pc[:], bk_sb[:, m : m + 1]
                        )
                        nc.vector.tensor_scalar_add(
                            ps[:], ps[:], bk_sb[:, 4 + m : 5 + m]
                        )
                    t1 = ktmp.tile([128, 512], f16, tag="kt1")
                    t2 = ktmp.tile([128, 512], f16, tag="kt2")
                    nc.vector.tensor_tensor(t1[:], pc[:], ck_sb[:, ls], ALU.mult)
                    nc.vector.tensor_tensor(t2[:], ps[:], sk_sb[:, ls], ALU.mult)
                    nc.vector.tensor_tensor(
                        kropeT[m][:, ls], t1[:], t2[:], ALU.add
                    )

                # v for the 4 l-tiles of this chunk (bf16 inputs; fp8 on the
                # v path costs too much output accuracy)
                for j in range(4):
                    lt = 4 * lch + j
                    pv = kpsum.tile([128, 512], f32, tag="vps")
                    for k in range(4):
                        nc.tensor.matmul(
                            pv[:],
                            r(ctxb_sb[:, k, 128 * lt : 128 * (lt + 1)]),
                            r(wv_sb[:, k, :]),
                            start=(k == 0),
                            stop=(k == 3),
                        )
                    if cfg["v_bias"]:
                        nc.vector.tensor_tensor(pv[:], pv[:], bv_sb[:], ALU.add)
                    va = vaug[lt][:].rearrange("p (h e) -> p h e", e=65)
                    nc.vector.tensor_copy(
                        va[:, :, 0:64],
                        pv[:].rearrange("p (h d) -> p h d", h=H),
                    )
                    nc.vector.memset(va[:, :, 64], 1.0)

        # ---- attention ---------------------------------------------------
        with tc.tile_pool(name="apsum", bufs=2, space="PSUM") as apsum, \
                tc.tile_pool(name="opsum", bufs=2, space="PSUM") as opsum, \
                tc.tile_pool(name="ptile", bufs=4) as ptile, \
                tc.tile_pool(name="stile", bufs=4) as stile:
            for tch in range(N_TCH):
                ts = slice(512 * tch, 512 * (tch + 1))
                for hp in range(4):
                    hA, hB = 2 * hp, 2 * hp + 1
                    oA = opsum.tile([65, 512], f32, tag="oA")
                    oB = opsum.tile([65, 512], f32, tag="oB")
                    for lt in range(N_LT):
                        lw = slice(128 * lt, 128 * (lt + 1))
                        g = apsum.tile([128, 1024], f32, tag="g")
                        nc.tensor.matmul(
                            g[:, 0:512],
                            r(kropeT[hp][0:64, lw]),
                            r(qropeT[hp][0:64, ts]),
                            start=True, stop=True,
                            tile_position=(0, 0),
                        )
                        nc.tensor.matmul(
                            g[:, 512:1024],
                            r(kropeT[hp][64:128, lw]),
                            r(qropeT[hp][64:128, ts]),
                            start=True, stop=True,
                            tile_position=(64, 0),
                        )
                        pT = ptile.tile([128, 1024], f16, tag="pT")
                        if cfg["kmask"]:
                            nc.scalar.activation(
                                pT[:], g[:], AF.Exp,
                                bias=kmb_sb[:, lt : lt + 1],
                                scale=1.0 / SCALE,
                            )
                        else:
                            nc.scalar.activation(
                                pT[:], g[:], AF.Exp, scale=1.0 / SCALE
                            )
                        nc.tensor.matmul(
                            oA[:],
                            r(vaug[lt][:, 65 * hA : 65 * hA + 65]),
                            r(pT[:, 0:512]),
                            start=(lt == 0), stop=(lt == N_LT - 1),
                        )
                        nc.tensor.matmul(
                            oB[:],
                            r(vaug[lt][:, 65 * hB : 65 * hB + 65]),
                            r(pT[:, 512:1024]),
                            start=(lt == 0), stop=(lt == N_LT - 1),
                        )
                    # evacuate unnormalized O; sums rows go to SBUF via DMA
                    # (engine copies cannot target unaligned partitions)
                    nc.vector.tensor_copy(
                        oraw[tch][hp][0:64, :], oA[0:64, :]
                    )
                    nc.vector.tensor_copy(
                        oraw[tch][hp][64:128, :], oB[0:64, :]
                    )
                    # stage the sums rows at partition 0, then DMA to row h
                    # (engine copies cannot target unaligned partitions)
                    for h, o in ((hA, oA), (hB, oB)):
                        srow = stile.tile([1, 512], f32, tag="srow")
                        nc.vector.tensor_copy(srow[:], o[64:65, :])
                        nc.sync.dma_start(sums_sb[tch][h : h + 1, :], srow[:])

        # ---- normalize + out projection ---------------------------------
        with tc.tile_pool(name="fpsum", bufs=2, space="PSUM") as fpsum, \
                tc.tile_pool(name="npsum", bufs=2, space="PSUM") as npsum, \
                tc.tile_pool(name="ftile", bufs=4) as ftile:
            for tch in range(N_TCH):
                ts = slice(512 * tch, 512 * (tch + 1))
                # batched softmax normalization: one reciprocal for 8 heads,
                # broadcast to head-pair partitions via a tiny select matmul
                rinv = ftile.tile([8, 512], f32, tag="rinv")
                nc.vector.reciprocal(rinv[:], sums_sb[tch][:])
                rinvh = ftile.tile([8, 512], f16, tag="rinvh")
                nc.vector.tensor_copy(rinvh[:], rinv[:])
                for hp in range(4):
                    rs = npsum.tile([128, 512], f32, tag="rs")
                    nc.tensor.matmul(
                        rs[:],
                        r(sel_sb[:, 128 * hp : 128 * (hp + 1)]),
                        r(rinvh[:]),
                        start=True, stop=True,
                    )
                    nc.vector.tensor_tensor(
                        onorm[tch][hp][:], oraw[tch][hp][:], rs[:], ALU.mult
                    )
                for m in range(4):
                    po = fpsum.tile([128, 512], f32, tag="po")
                    for hp in range(4):
                        nc.tensor.matmul(
                            po[:],
                            r(wot_sb[hp][:, 128 * m : 128 * (m + 1)]),
                            r(onorm[tch][hp][:]),
                            start=(hp == 0), stop=(hp == 3),
                        )
                    ob = ftile.tile([128, 512], f32, tag="ob")
                    # add bo (per-partition scalar broadcast along t)
                    nc.vector.tensor_scalar_add(ob[:], po[:], bo_sb[:, m : m + 1])
                    nc.sync.dma_start(out_re[:, m, ts], ob[:])

    nc.finalize()
    return nc


# ---------------------------------------------------------------------------
# Host-side input prep per core
# ---------------------------------------------------------------------------


def _head_swap_perm():
    a = np.arange(ATT)
    h = a // HD
    j = a % HD
    return h * HD + (j + 32) % HD


def _rope_tables(pos, length, n, scale):
    theta = ROPE_GAMMA / 10000.0 ** (np.arange(0, HD, 2, dtype=np.float64) / HD)
    f = pos[None, :].astype(np.float64) / max(float(length), 1e-30) * theta[:, None]
    c32 = (np.cos(f) * scale).astype(np.float32)  # [32, n]
    s32 = (np.sin(f) * scale).astype(np.float32)
    chalf = np.concatenate([c32, c32], axis=0)  # [64, n]
    shalf = np.concatenate([-s32, s32], axis=0)
    ctab = np.concatenate([chalf, chalf], axis=0)  # [128, n] (2 heads)
    stab = np.concatenate([shalf, shalf], axis=0)
    return np.ascontiguousarray(ctab), np.ascontiguousarray(stab)


def _to_f8(a):
    import ml_dtypes

    return np.ascontiguousarray(
        np.clip(a, -240.0, 240.0).astype(ml_dtypes.float8_e4m3)
    )


def _to_bf16(a):
    import ml_dtypes

    return np.ascontiguousarray(a.astype(ml_dtypes.bfloat16))


def _to_f16(a):
    return np.ascontiguousarray(a.astype(np.float16))


def _prep_core_inputs(core, x, context, x_mask, context_mask,
                      Wq, bq, Wk, bk, Wv, bv, Wo, bo, cfg):
    b = core // 2
    th = core % 2
    t0 = th * T_CORE
    perm = _head_swap_perm()

    wqt = Wq.T * W8_SCALE
    wkt = Wk.T * W8_SCALE
    ctxT = np.ascontiguousarray(context[b].T)
    m = {
        "x8": _to_f8(x[b][:, t0 : t0 + T_CORE]),
        "ctx8": _to_f8(ctxT),
        "wqt": _to_f8(wqt),
        "wqts": _to_f8(wqt[:, perm]),
        "wkt": _to_f8(wkt),
        "wkts": _to_f8(wkt[:, perm]),
        "ctxb": _to_bf16(ctxT),
        "wvt": _to_bf16(Wv.T),
        "wot": _to_f16(Wo.T),
        "bo": np.ascontiguousarray(bo.astype(np.float32)),
    }
    selm = np.zeros((8, 512), np.float32)
    for hp in range(4):
        selm[2 * hp, 128 * hp : 128 * hp + 64] = 1.0
        selm[2 * hp + 1, 128 * hp + 64 : 128 * hp + 128] = 1.0
    m["sel"] = _to_f16(selm)

    len_q = float(x_mask[b].sum())
    len_k = float(context_mask[b].sum())
    # 1/W8_SCALE descale of the fp8 projection folded into the tables
    cqt, sqt = _rope_tables(
        np.arange(t0, t0 + T_CORE), len_q, T_CORE, 1.0 / W8_SCALE
    )
    ckt, skt = _rope_tables(np.arange(L), len_k, L, 1.0 / W8_SCALE)
    m["cq"], m["sq"] = _to_f16(cqt), _to_f16(sqt)
    m["ck"], m["sk"] = _to_f16(ckt), _to_f16(skt)

    if cfg["qk_bias"]:
        # per-partition bias values, pre-scaled to match the fp8 psum
        # (psum holds W8_SCALE * (Wx); tables divide by W8_SCALE, so the
        # bias added in psum must also be scaled by W8_SCALE)
        bqv = np.zeros((128, 8), np.float32)
        bkv = np.zeros((128, 8), np.float32)
        for mt in range(4):
            bqv[:, mt] = bq[128 * mt : 128 * (mt + 1)] * W8_SCALE
            bqv[:, 4 + mt] = bq[perm][128 * mt : 128 * (mt + 1)] * W8_SCALE
            bkv[:, mt] = bk[128 * mt : 128 * (mt + 1)] * W8_SCALE
            bkv[:, 4 + mt] = bk[perm][128 * mt : 128 * (mt + 1)] * W8_SCALE
        m["bqv"], m["bkv"] = bqv, bkv
    if cfg["v_bias"]:
        m["bvt"] = np.ascontiguousarray(
            np.broadcast_to(bv[None, :], (128, ATT)).astype(np.float32)
        )
    if cfg["kmask"]:
        cm = context_mask[b].reshape(L)
        kmb = np.zeros((128, N_LT), np.float32)
        for lt in range(N_LT):
            kmb[:, lt] = np.where(cm[128 * lt : 128 * (lt + 1)] == 0, MASK_NEG, 0.0)
        m["kmb"] = kmb
    return m


def core_slices(c):
    """Index into the full [B, D_MODEL, T] output owned by core c."""
    b, th = c // 2, c % 2
    return (b, slice(None), slice(th * T_CORE, (th + 1) * T_CORE))


def kernel(**inputs):
    from concourse.bass_utils import run_bass_kernel_spmd

    x = np.asarray(inputs["x"], np.float32)
    context = np.asarray(inputs["context"], np.float32)
    x_mask = np.asarray(inputs["x_mask"], np.float32)
    context_mask = np.asarray(inputs["context_mask"], np.float32)
    args = dict(
        x=x, context=context, x_mask=x_mask, context_mask=context_mask,
        Wq=np.asarray(inputs["Wq"], np.float32),
        bq=np.asarray(inputs["bq"], np.float32),
        Wk=np.asarray(inputs["Wk"], np.float32),
        bk=np.asarray(inputs["bk"], np.float32),
        Wv=np.asarray(inputs["Wv"], np.float32),
        bv=np.asarray(inputs["bv"], np.float32),
        Wo=np.asarray(inputs["Wo"], np.float32),
        bo=np.asarray(inputs["bo"], np.float32),
    )

    cfg = {
        "qk_bias": bool(np.any(args["bq"]) or np.any(args["bk"])),
        "v_bias": bool(np.any(args["bv"])),
        "kmask": bool(np.any(context_mask == 0)),
    }

    nc = _build_nc(cfg)
    in_maps = [_prep_core_inputs(c, cfg=cfg, **args) for c in range(N_CORES)]
    res = run_bass_kernel_spmd(nc, in_maps, list(range(N_CORES)))

    out = np.empty((B, D_MODEL, T), np.float32)
    for c in range(N_CORES):
        out[core_slices(c)] = res.results[c]["out"]
    # x_mask gate (exact; all-ones in this problem)
    out = out * x_mask  # [B,1,T] broadcasts over D_MODEL
    return out


# revision 35
# speedup vs baseline: 1.2282x; 1.0468x over previous
# Trainium2 Bass kernel for nn_AttentionModule_16011638080155.
#
# Cross-attention with length-normalized RoPE, softmax over context L,
# out-projection, written as [B, D_MODEL, T].
#
# Sharding: 8 cores = (batch b in 0..4) x (T half in 0..2). Each core computes
# its full attention output slice [D_MODEL, 1024] independently (k/v projection
# duplicated across the two T-halves of a batch; no collectives).
#
# Device layout (per core), everything "S-transposed" so softmax output feeds
# the PV matmul with no transposes:
#   q.T  [a=512, t=1024]  = WqT.T @ x        (fp8 DoubleRow, weights x256)
#   k.T  [a=512, l=2048]  = WkT.T @ ctxT    (fp8 DoubleRow, weights x256)
#   v    [l=2048, a=512]  = ctxT.T @ WvT    (bf16; stored per l-tile, ones-aug)
#   S.T  [l, t]           = k_rope.T x q_rope (f16, per head, row-tiled pairs)
#   P.T  = exp(S.T / sqrt(512))              (ACT, per-partition mask bias)
#   O    [65, t] = [V | 1s].T @ P.T          (row 64 = softmax sums)
#   out  [dm, t] = WoT.T @ (O * recip(sums)) (+bo)
#
# The exp stream on the scalar engine (~142us) is the phase floor; Q/K
# projection chunks are interleaved into the attention emission so the PE
# fills exp-wait bubbles and the scalar engine starts early. All DRAM
# parameters are host-side pre-arranged to the SBUF tile layouts so every
# DMA is contiguous per partition (few descriptors).
import math

import numpy as np

# ---------------------------------------------------------------------------
# Workaround for walrus CoreV2/V3 "Too many sync wait commands" on the Tile
# kernel-tail drain: move the accumulated sem waits off the single Drain
# instruction onto preceding nop instructions (same engine, in-order), at
# most 1 wait per instruction.
# ---------------------------------------------------------------------------


def _install_tile_drain_patch():
    import concourse.mybir as mybir
    import concourse.tile as tile_mod
    from concourse.vector_clock import ScopedClock

    if getattr(tile_mod.TileContext, "_drain_patch_installed", False):
        return

    def _patched_drain_and_barrier(self, tick_clock, wait_clock):
        nc = self.nc
        sink = nc.sync.nop(nofuse=True)
        wait_clock.add_sem_waits(
            sink.ins, ScopedClock({None: tick_clock.global_clock})
        )
        si = sink.ins.sync_info
        waits = list(si.on_wait) if si is not None else []
        if len(waits) > 1:
            sink.ins.sync_info = mybir.SyncInfo(on_wait=waits[:1], on_update=[])
            rest = waits[1:]
            for i in range(len(rest)):
                n2 = nc.sync.nop(nofuse=True)
                n2.ins.sync_info = mybir.SyncInfo(
                    on_wait=rest[i : i + 1], on_update=[]
                )
        nc.sync.drain()

        nc.all_engine_barrier()
        assert self.sems is not None
        popped = nc._tile_sem_poison_stack.pop()
        assert popped is self._sem_poison
        nc.clear_and_free_semaphores(list(self.sems.allocated().values()))
        nc.all_engine_barrier()

    tile_mod.TileContext._drain_and_barrier = _patched_drain_and_barrier
    tile_mod.TileContext._drain_patch_installed = True


# ---------------------------------------------------------------------------
# Problem constants (hardcoded per the harness contract).
# ---------------------------------------------------------------------------
B = 4
D_MODEL = 512
T = 2048
L = 2048
D_CTX = 512
ATT = 512
H = 8
HD = 64
ROPE_GAMMA = 10.0
SCALE = math.sqrt(ATT)

N_CORES = 8
T_CORE = T // 2  # 1024, each core handles half the query positions
N_TCH = T_CORE // 512  # 2 chunks of 512
N_LCH = L // 512  # 4
N_LT = L // 128  # 16
MASK_NEG = -60.0  # applied post-scale inside exp(); exp(-60) ~ 8.8e-27
W8_SCALE = 256.0  # fp8 weight scale; descale folded into rope tables


def _build_nc(cfg):
    """Build the single-core Bass program (same program runs SPMD on 8 cores)."""
    import concourse.bacc as bacc
    import concourse.mybir as mybir
    import concourse.tile as tile
    from contextlib import ExitStack

    _install_tile_drain_patch()

    f32 = mybir.dt.float32
    f16 = mybir.dt.float16
    bf16 = mybir.dt.bfloat16
    f8 = mybir.dt.float8e4
    AF = mybir.ActivationFunctionType
    ALU = mybir.AluOpType
    DR = mybir.MatmulPerfMode.DoubleRow

    def r(ap):
        return ap

    nc = bacc.Bacc("TRN2", target_bir_lowering=False, debug=False)

    # ---- DRAM parameters (host pre-arranged to SBUF layouts) -------------
    x8 = nc.declare_dram_parameter("x8", [128, 4 * T_CORE], f8, isOutput=False)
    ctx8 = nc.declare_dram_parameter("ctx8", [128, 4 * L], f8, isOutput=False)
    wqt = nc.declare_dram_parameter("wqt", [128, 4 * ATT], f8, isOutput=False)
    wqts = nc.declare_dram_parameter("wqts", [128, 4 * ATT], f8, isOutput=False)
    wkt = nc.declare_dram_parameter("wkt", [128, 4 * ATT], f8, isOutput=False)
    wkts = nc.declare_dram_parameter("wkts", [128, 4 * ATT], f8, isOutput=False)
    ctxb = nc.declare_dram_parameter("ctxb", [128, 4 * L], bf16, isOutput=False)
    wvt = nc.declare_dram_parameter("wvt", [128, 4 * ATT], bf16, isOutput=False)
    wot = nc.declare_dram_parameter("wot", [ATT, D_MODEL], f16, isOutput=False)
    cq = nc.declare_dram_parameter("cq", [128, T_CORE], f16, isOutput=False)
    sq = nc.declare_dram_parameter("sq", [128, T_CORE], f16, isOutput=False)
    ck = nc.declare_dram_parameter("ck", [128, L], f16, isOutput=False)
    sk = nc.declare_dram_parameter("sk", [128, L], f16, isOutput=False)
    bo = nc.declare_dram_parameter("bo", [128, 4], f32, isOutput=False)
    # broadcast selector: sel[h, 128*hp + c] = 1 iff h == 2*hp + (c >= 64)
    sel = nc.declare_dram_parameter("sel", [8, 512], f16, isOutput=False)
    if cfg["qk_bias"]:
        # per-partition bias values: columns [bq(4 m-tiles) | swapped(4)]
        bqv = nc.declare_dram_parameter("bqv", [128, 8], f32, isOutput=False)
        bkv = nc.declare_dram_parameter("bkv", [128, 8], f32, isOutput=False)
    if cfg["v_bias"]:
        bvt = nc.declare_dram_parameter("bvt", [128, ATT], f32, isOutput=False)
    if cfg["kmask"]:
        kmb = nc.declare_dram_parameter("kmb", [128, N_LT], f32, isOutput=False)
    out = nc.declare_dram_parameter("out", [D_MODEL, T_CORE], f32, isOutput=True)

    out_re = out.rearrange("(kp p) t -> p kp t", p=128)

    with tile.TileContext(nc) as tc, ExitStack() as ctx:
        # ---- persistent SBUF tiles --------------------------------------
        per = ctx.enter_context(tc.tile_pool(name="per", bufs=1))
        qropeT = [per.tile([128, T_CORE], f16, tag=f"qrope{m}", name=f"qrope{m}") for m in range(4)]
        kropeT = [per.tile([128, L], f16, tag=f"krope{m}", name=f"krope{m}") for m in range(4)]
        vaug = [per.tile([128, H * 65], f16, tag=f"vaug{lt}", name=f"vaug{lt}") for lt in range(N_LT)]
        oraw = [
            [per.tile([128, 512], f16, tag=f"or{tch}_{hp}", name=f"or{tch}_{hp}") for hp in range(4)]
            for tch in range(N_TCH)
        ]
        onorm = [
            [per.tile([128, 512], f16, tag=f"on{tch}_{hp}", name=f"on{tch}_{hp}") for hp in range(4)]
            for tch in range(N_TCH)
        ]
        sums_sb = [
            per.tile([8, 512], f32, tag=f"sums{tch}", name=f"sums{tch}")
            for tch in range(N_TCH)
        ]
        wot_sb = [per.tile([128, D_MODEL], f16, tag=f"wot{hp}", name=f"wot{hp}") for hp in range(4)]
        bo_sb = per.tile([128, 4], f32, tag="bo")
        if cfg["kmask"]:
            kmb_sb = per.tile([128, N_LT], f32, tag="kmb")

        # input staging tiles (persistent so all loads prefetch up front)
        x_sb = per.tile([128, 4, T_CORE], f8, tag="x")
        wq_sb = per.tile([128, 4, ATT], f8, tag="wq")
        wqs_sb = per.tile([128, 4, ATT], f8, tag="wqs")
        cq_sb = per.tile([128, T_CORE], f16, tag="cq")
        sq_sb = per.tile([128, T_CORE], f16, tag="sq")
        wk_sb = per.tile([128, 4, ATT], f8, tag="wk")
        wks_sb = per.tile([128, 4, ATT], f8, tag="wks")
        wv_sb = per.tile([128, 4, ATT], bf16, tag="wv")
        ctx8_sb = per.tile([128, 4, L], f8, tag="ctx8")
        ctxb_sb = per.tile([128, 4, L], bf16, tag="ctxb")
        ck_sb = per.tile([128, L], f16, tag="ck")
        sk_sb = per.tile([128, L], f16, tag="sk")

        # prefetch in consumption order: Q chunk 0, K chunk 0, V, rest
        nc.sync.dma_start(x_sb[:], x8.rearrange("p (k n) -> p k n", k=4))
        nc.sync.dma_start(wq_sb[:], wqt.rearrange("p (k n) -> p k n", k=4))
        nc.sync.dma_start(wqs_sb[:], wqts.rearrange("p (k n) -> p k n", k=4))
        nc.sync.dma_start(cq_sb[:], cq[:])
        nc.sync.dma_start(sq_sb[:], sq[:])
        nc.sync.dma_start(ctx8_sb[:], ctx8.rearrange("p (k n) -> p k n", k=4))
        nc.sync.dma_start(wk_sb[:], wkt.rearrange("p (k n) -> p k n", k=4))
        nc.sync.dma_start(wks_sb[:], wkts.rearrange("p (k n) -> p k n", k=4))
        nc.sync.dma_start(ck_sb[:], ck[:])
        nc.sync.dma_start(sk_sb[:], sk[:])
        nc.sync.dma_start(ctxb_sb[:], ctxb.rearrange("p (k n) -> p k n", k=4))
        nc.sync.dma_start(wv_sb[:], wvt.rearrange("p (k n) -> p k n", k=4))
        for hp in range(4):
            nc.sync.dma_start(wot_sb[hp][:], wot[128 * hp : 128 * (hp + 1), :])
        nc.sync.dma_start(bo_sb[:], bo[:])
        sel_sb = per.tile([8, 512], f16, tag="sel")
        nc.sync.dma_start(sel_sb[:], sel[:])
        if cfg["kmask"]:
            nc.sync.dma_start(kmb_sb[:], kmb[:])
        if cfg["qk_bias"]:
            bq_sb = per.tile([128, 8], f32, tag="bq")
            bk_sb = per.tile([128, 8], f32, tag="bk")
            nc.sync.dma_start(bq_sb[:], bqv[:])
            nc.sync.dma_start(bk_sb[:], bkv[:])
        if cfg["v_bias"]:
            bv_sb = per.tile([128, ATT], f32, tag="bv")
            nc.sync.dma_start(bv_sb[:], bvt[:])

        ptmp = ctx.enter_context(tc.tile_pool(name="ptmp", bufs=4))

        # psum pool lifetimes are managed as a stack so projection chunks can
        # interleave with attention under the 8-bank budget: apsum(4)+opsum(2)
        # at the bottom span the whole attention; projpsum(2) on top spans the
        # Q/K/V chunks and is then swapped for fpsum(2) (normalize + out).
        attn_es = ExitStack()
        apsum = attn_es.enter_context(
            tc.tile_pool(name="apsum", bufs=2, space="PSUM")
        )
        opsum = attn_es.enter_context(
            tc.tile_pool(name="opsum", bufs=1, space="PSUM")
        )
        ptile = attn_es.enter_context(tc.tile_pool(name="ptile", bufs=4))
        stile = attn_es.enter_context(tc.tile_pool(name="stile", bufs=4))

        proj_es = ExitStack()
        projpsum = proj_es.enter_context(
            tc.tile_pool(name="projpsum", bufs=1, space="PSUM")
        )

        def q_chunk(m):
            """qropeT[m] for both tch halves (fp8 DoubleRow + rope)."""
            for tch in range(N_TCH):
                ts = slice(512 * tch, 512 * (tch + 1))
                pc = projpsum.tile([128, 512], f32, tag="pc")
                ps = projpsum.tile([128, 512], f32, tag="ps")
                for i in range(2):
                    nc.tensor.matmul(
                        pc[:],
                        r(wq_sb[:, 2 * i : 2 * i + 2, 128 * m : 128 * (m + 1)]),
                        r(x_sb[:, 2 * i : 2 * i + 2, ts]),
                        start=(i == 0), stop=(i == 1), perf_mode=DR,
                    )
                    nc.tensor.matmul(
                        ps[:],
                        r(wqs_sb[:, 2 * i : 2 * i + 2, 128 * m : 128 * (m + 1)]),
                        r(x_sb[:, 2 * i : 2 * i + 2, ts]),
                        start=(i == 0), stop=(i == 1), perf_mode=DR,
                    )
                if cfg["qk_bias"]:
                    nc.vector.tensor_scalar_add(pc[:], pc[:], bq_sb[:, m : m + 1])
                    nc.vector.tensor_scalar_add(ps[:], ps[:], bq_sb[:, 4 + m : 5 + m])
                t1 = ptmp.tile([128, 512], f16, tag="t1")
                t2 = ptmp.tile([128, 512], f16, tag="t2")
                nc.vector.tensor_tensor(t1[:], pc[:], cq_sb[:, ts], ALU.mult)
                nc.vector.tensor_tensor(t2[:], ps[:], sq_sb[:, ts], ALU.mult)
                nc.vector.tensor_tensor(qropeT[m][:, ts], t1[:], t2[:], ALU.add)

        def k_chunk(m):
            """kropeT[m] over all l chunks (fp8 DoubleRow + rope)."""
            for lch in range(N_LCH):
                ls = slice(512 * lch, 512 * (lch + 1))
                pc = projpsum.tile([128, 512], f32, tag="pc")
                ps = projpsum.tile([128, 512], f32, tag="ps")
                for i in range(2):
                    nc.tensor.matmul(
                        pc[:],
                        r(wk_sb[:, 2 * i : 2 * i + 2, 128 * m : 128 * (m + 1)]),
                        r(ctx8_sb[:, 2 * i : 2 * i + 2, ls]),
                        start=(i == 0), stop=(i == 1), perf_mode=DR,
                    )
                    nc.tensor.matmul(
                        ps[:],
                        r(wks_sb[:, 2 * i : 2 * i + 2, 128 * m : 128 * (m + 1)]),
                        r(ctx8_sb[:, 2 * i : 2 * i + 2, ls]),
                        start=(i == 0), stop=(i == 1), perf_mode=DR,
                    )
                if cfg["qk_bias"]:
                    nc.vector.tensor_scalar_add(pc[:], pc[:], bk_sb[:, m : m + 1])
                    nc.vector.tensor_scalar_add(ps[:], ps[:], bk_sb[:, 4 + m : 5 + m])
                t1 = ptmp.tile([128, 512], f16, tag="kt1")
                t2 = ptmp.tile([128, 512], f16, tag="kt2")
                nc.vector.tensor_tensor(t1[:], pc[:], ck_sb[:, ls], ALU.mult)
                nc.vector.tensor_tensor(t2[:], ps[:], sk_sb[:, ls], ALU.mult)
                nc.vector.tensor_tensor(kropeT[m][:, ls], t1[:], t2[:], ALU.add)

        # ---- Q0, K0, V (V ping-pongs through the projpsum banks) --------
        q_chunk(0)
        k_chunk(0)
        for lt in range(N_LT):
            pv = projpsum.tile([128, 512], f32, tag="pc" if lt % 2 == 0 else "ps")
            for k in range(4):
                nc.tensor.matmul(
                    pv[:],
                    r(ctxb_sb[:, k, 128 * lt : 128 * (lt + 1)]),
                    r(wv_sb[:, k, :]),
                    start=(k == 0),
                    stop=(k == 3),
                )
            if cfg["v_bias"]:
                nc.vector.tensor_tensor(pv[:], pv[:], bv_sb[:], ALU.add)
            va = vaug[lt][:].rearrange("p (h e) -> p h e", e=65)
            nc.vector.tensor_copy(
                va[:, :, 0:64],
                pv[:].rearrange("p (h d) -> p h d", h=H),
            )
            nc.vector.memset(va[:, :, 64], 1.0)

        # ---- attention (+ interleaved projection chunks) ----------------
        def attn(tch, hp):
            ts = slice(512 * tch, 512 * (tch + 1))
            hA, hB = 2 * hp, 2 * hp + 1
            oA = opsum.tile([65, 512], f32, tag="oA")
            oB = opsum.tile([65, 512], f32, tag="oB")
            for lt in range(N_LT):
                lw = slice(128 * lt, 128 * (lt + 1))
                g = apsum.tile([128, 1024], f32, tag="g")
                nc.tensor.matmul(
                    g[:, 0:512],
                    r(kropeT[hp][0:64, lw]),
                    r(qropeT[hp][0:64, ts]),
                    start=True, stop=True,
                    tile_position=(0, 0),
                )
                nc.tensor.matmul(
                    g[:, 512:1024],
                    r(kropeT[hp][64:128, lw]),
                    r(qropeT[hp][64:128, ts]),
                    start=True, stop=True,
                    tile_position=(64, 0),
                )
                pT = ptile.tile([128, 1024], f16, tag="pT")
                if cfg["kmask"]:
                    nc.scalar.activation(
                        pT[:], g[:], AF.Exp,
                        bias=kmb_sb[:, lt : lt + 1],
                        scale=1.0 / SCALE,
                    )
                else:
                    nc.scalar.activation(pT[:], g[:], AF.Exp, scale=1.0 / SCALE)
                nc.tensor.matmul(
                    oA[:],
                    r(vaug[lt][:, 65 * hA : 65 * hA + 65]),
                    r(pT[:, 0:512]),
                    start=(lt == 0), stop=(lt == N_LT - 1),
                )
                nc.tensor.matmul(
                    oB[:],
                    r(vaug[lt][:, 65 * hB : 65 * hB + 65]),
                    r(pT[:, 512:1024]),
                    start=(lt == 0), stop=(lt == N_LT - 1),
                )
            # evacuate unnormalized O; sums rows staged at partition 0 then
            # DMAed to row h (engine copies cannot shift to odd partitions)
            nc.vector.tensor_copy(oraw[tch][hp][0:64, :], oA[0:64, :])
            nc.vector.tensor_copy(oraw[tch][hp][64:128, :], oB[0:64, :])
            for h, o in ((hA, oA), (hB, oB)):
                srow = stile.tile([1, 512], f32, tag="srow")
                nc.vector.tensor_copy(srow[:], o[64:65, :])
                nc.sync.dma_start(sums_sb[tch][h : h + 1, :], srow[:])

        def normout(tch, fpsum, ftile):
            ts = slice(512 * tch, 512 * (tch + 1))
            rinv = ftile.tile([8, 512], f32, tag="rinv")
            nc.vector.reciprocal(rinv[:], sums_sb[tch][:])
            rinvh = ftile.tile([8, 512], f16, tag="rinvh")
            nc.vector.tensor_copy(rinvh[:], rinv[:])
            for hp in range(4):
                rs = fpsum.tile([128, 512], f32, tag="rs")
                nc.tensor.matmul(
                    rs[:],
                    r(sel_sb[:, 128 * hp : 128 * (hp + 1)]),
                    r(rinvh[:]),
                    start=True, stop=True,
                )
                nc.vector.tensor_tensor(
                    onorm[tch][hp][:], oraw[tch][hp][:], rs[:], ALU.mult
                )
            for m in range(4):
                po = fpsum.tile([128, 512], f32, tag="po")
                for hp in range(4):
                    nc.tensor.matmul(
                        po[:],
                        r(wot_sb[hp][:, 128 * m : 128 * (m + 1)]),
                        r(onorm[tch][hp][:]),
                        start=(hp == 0), stop=(hp == 3),
                    )
                ob = ftile.tile([128, 512], f32, tag="ob")
                nc.vector.tensor_scalar_add(ob[:], po[:], bo_sb[:, m : m + 1])
                nc.sync.dma_start(out_re[:, m, ts], ob[:])

        # tch 0 attention with Q/K chunks slotted between head pairs
        for hp in range(4):
            attn(0, hp)
            if hp < 3:
                q_chunk(hp + 1)
                k_chunk(hp + 1)
        proj_es.close()  # free projpsum banks for fpsum

        fin_es = ExitStack()
        fpsum = fin_es.enter_context(
            tc.tile_pool(name="fpsum", bufs=1, space="PSUM")
        )
        ftile = fin_es.enter_context(tc.tile_pool(name="ftile", bufs=4))

        normout(0, fpsum, ftile)
        for hp in range(4):
            attn(1, hp)
        normout(1, fpsum, ftile)

        fin_es.close()
        attn_es.close()

    nc.finalize()
    return nc


# ---------------------------------------------------------------------------
# Host-side input prep per core
# ---------------------------------------------------------------------------


def _head_swap_perm():
    a = np.arange(ATT)
    h = a // HD
    j = a % HD
    return h * HD + (j + 32) % HD


def _rope_tables(pos, length, n, scale):
    theta = ROPE_GAMMA / 10000.0 ** (np.arange(0, HD, 2, dtype=np.float64) / HD)
    f = pos[None, :].astype(np.float64) / max(float(length), 1e-30) * theta[:, None]
    c32 = (np.cos(f) * scale).astype(np.float32)  # [32, n]
    s32 = (np.sin(f) * scale).astype(np.float32)
    chalf = np.concatenate([c32, c32], axis=0)  # [64, n]
    shalf = np.concatenate([-s32, s32], axis=0)
    ctab = np.concatenate([chalf, chalf], axis=0)  # [128, n] (2 heads)
    stab = np.concatenate([shalf, shalf], axis=0)
    return np.ascontiguousarray(ctab), np.ascontiguousarray(stab)


def _fold128(a):
    """[512, N] -> [128, 4*N]: partition-major fold to the SBUF tile layout."""
    n = a.shape[1]
    return np.ascontiguousarray(
        a.reshape(4, 128, n).transpose(1, 0, 2).reshape(128, 4 * n)
    )


def _to_f8(a):
    import ml_dtypes

    return np.ascontiguousarray(
        np.clip(a, -240.0, 240.0).astype(ml_dtypes.float8_e4m3)
    )


def _to_bf16(a):
    import ml_dtypes

    return np.ascontiguousarray(a.astype(ml_dtypes.bfloat16))


def _to_f16(a):
    return np.ascontiguousarray(a.astype(np.float16))


def _prep_core_inputs(core, x, context, x_mask, context_mask,
                      Wq, bq, Wk, bk, Wv, bv, Wo, bo, cfg):
    b = core // 2
    th = core % 2
    t0 = th * T_CORE
    perm = _head_swap_perm()

    wqt = Wq.T * W8_SCALE
    wkt = Wk.T * W8_SCALE
    ctxT = np.ascontiguousarray(context[b].T)
    m = {
        "x8": _to_f8(_fold128(x[b][:, t0 : t0 + T_CORE])),
        "ctx8": _to_f8(_fold128(ctxT)),
        "ctxb": _to_bf16(_fold128(ctxT)),
        "wqt": _to_f8(_fold128(wqt)),
        "wqts": _to_f8(_fold128(wqt[:, perm])),
        "wkt": _to_f8(_fold128(wkt)),
        "wkts": _to_f8(_fold128(wkt[:, perm])),
        "wvt": _to_bf16(_fold128(Wv.T)),
        "wot": _to_f16(Wo.T),
        "bo": np.ascontiguousarray(
            bo.astype(np.float32).reshape(4, 128).T
        ),
    }
    selm = np.zeros((8, 512), np.float32)
    for hp in range(4):
        selm[2 * hp, 128 * hp : 128 * hp + 64] = 1.0
        selm[2 * hp + 1, 128 * hp + 64 : 128 * hp + 128] = 1.0
    m["sel"] = _to_f16(selm)

    len_q = float(x_mask[b].sum())
    len_k = float(context_mask[b].sum())
    # 1/W8_SCALE descale of the fp8 projection folded into the tables
    cqt, sqt = _rope_tables(
        np.arange(t0, t0 + T_CORE), len_q, T_CORE, 1.0 / W8_SCALE
    )
    ckt, skt = _rope_tables(np.arange(L), len_k, L, 1.0 / W8_SCALE)
    m["cq"], m["sq"] = _to_f16(cqt), _to_f16(sqt)
    m["ck"], m["sk"] = _to_f16(ckt), _to_f16(skt)

    if cfg["qk_bias"]:
        # per-partition bias values, pre-scaled to match the fp8 psum
        bqv = np.zeros((128, 8), np.float32)
        bkv = np.zeros((128, 8), np.float32)
        for mt in range(4):
            bqv[:, mt] = bq[128 * mt : 128 * (mt + 1)] * W8_SCALE
            bqv[:, 4 + mt] = bq[perm][128 * mt : 128 * (mt + 1)] * W8_SCALE
            bkv[:, mt] = bk[128 * mt : 128 * (mt + 1)] * W8_SCALE
            bkv[:, 4 + mt] = bk[perm][128 * mt : 128 * (mt + 1)] * W8_SCALE
        m["bqv"], m["bkv"] = bqv, bkv
    if cfg["v_bias"]:
        m["bvt"] = np.ascontiguousarray(
            np.broadcast_to(bv[None, :], (128, ATT)).astype(np.float32)
        )
    if cfg["kmask"]:
        cm = context_mask[b].reshape(L)
        kmb = np.zeros((128, N_LT), np.float32)
        for lt in range(N_LT):
            kmb[:, lt] = np.where(cm[128 * lt : 128 * (lt + 1)] == 0, MASK_NEG, 0.0)
        m["kmb"] = kmb
    return m


def core_slices(c):
    """Index into the full [B, D_MODEL, T] output owned by core c."""
    b, th = c // 2, c % 2
    return (b, slice(None), slice(th * T_CORE, (th + 1) * T_CORE))


def kernel(**inputs):
    from concourse.bass_utils import run_bass_kernel_spmd

    x = np.asarray(inputs["x"], np.float32)
    context = np.asarray(inputs["context"], np.float32)
    x_mask = np.asarray(inputs["x_mask"], np.float32)
    context_mask = np.asarray(inputs["context_mask"], np.float32)
    args = dict(
        x=x, context=context, x_mask=x_mask, context_mask=context_mask,
        Wq=np.asarray(inputs["Wq"], np.float32),
        bq=np.asarray(inputs["bq"], np.float32),
        Wk=np.asarray(inputs["Wk"], np.float32),
        bk=np.asarray(inputs["bk"], np.float32),
        Wv=np.asarray(inputs["Wv"], np.float32),
        bv=np.asarray(inputs["bv"], np.float32),
        Wo=np.asarray(inputs["Wo"], np.float32),
        bo=np.asarray(inputs["bo"], np.float32),
    )

    cfg = {
        "qk_bias": bool(np.any(args["bq"]) or np.any(args["bk"])),
        "v_bias": bool(np.any(args["bv"])),
        "kmask": bool(np.any(context_mask == 0)),
    }

    nc = _build_nc(cfg)
    in_maps = [_prep_core_inputs(c, cfg=cfg, **args) for c in range(N_CORES)]
    res = run_bass_kernel_spmd(nc, in_maps, list(range(N_CORES)))

    out = np.empty((B, D_MODEL, T), np.float32)
    for c in range(N_CORES):
        out[core_slices(c)] = res.results[c]["out"]
    # x_mask gate (exact; all-ones in this problem)
    out = out * x_mask  # [B,1,T] broadcasts over D_MODEL
    return out
